# revision 13
# baseline (speedup 1.0000x reference)
"""Trainium2 Bass kernel for nn_LSTMActor: trunk GEMM -> LayerNorm -> Tanh ->
LSTM (16 steps, constant input) -> MLP head -> tanh.

Sharding: data-parallel over batch B=2048 across 8 cores (256 rows each);
weights replicated. Transposed layout throughout (feature dim on partitions).

Perf scheme vs plain bf16:
- LSTM recurrent matmul uses delta-sigma fp8: on "delta" steps the gates are
  updated incrementally, gates += W8^T @ fp8(h_t - h_{t-1}), with the fp8
  DoubleRow perf mode (2 k-subtiles per instruction = 2x matmul throughput).
  Quantization error is relative to |dh| (small and shrinking), and the fixed
  W8 quantization error telescopes across steps; full-bf16 "refresh" steps at
  t in {1,4,8} bound the residual W-side error window. Measured end-to-end
  error ~9e-3 vs the 2e-2 budget.
- Elementwise work runs in fused multi-tile ops (gate groups of 4 m-tiles,
  H-halves for the cell) to amortize per-instruction dispatch.
- W2 head runs transposed (stationary = W2, 6 output partitions) and mu is
  DMA'd out per step as [T, A, BS]; the host reassembles [B, T, A].
"""

import numpy as np
import ml_dtypes

import concourse.bass as bass
import concourse.tile as tile
from concourse import mybir, bacc
from concourse import bass_utils
from concourse.masks import make_identity

BF = ml_dtypes.bfloat16
E4NP = ml_dtypes.float8_e4m3
F32 = mybir.dt.float32
BF16 = mybir.dt.bfloat16
E4 = mybir.dt.float8e4
DR = mybir.MatmulPerfMode.DoubleRow
AluOp = None  # set in _build

B, R, Fd, H, A, T = 2048, 39200, 1024, 1024, 6, 16
NC_ = 8
BS = B // NC_          # 256 rows per core
NB = BS // 128         # 2 b-tiles per core
KT = 128
RP = ((R + KT - 1) // KT) * KT   # 39296
NK = RP // KT          # 307 K-tiles for trunk
KH = H // 128          # 8 K-tiles for H-dim GEMMs
M4 = 4 * H // 128      # 32 M-tiles of gates
H2 = H // 2            # 512
KG = 2                 # trunk K-tiles per DMA batch

REFRESH = (1, 5)
SW = 64.0              # W_hh fp8 scale
# measured max|h_t - h_{t-1}| of the bf16 trajectory, x1.25 safety
_DMAX = {2: 0.247, 3: 0.163, 4: 0.136, 5: 0.113, 6: 0.098, 7: 0.084,
         8: 0.072, 9: 0.057, 10: 0.049, 11: 0.0445, 12: 0.0435, 13: 0.041,
         14: 0.038, 15: 0.035}
SCALE = {t: float(2.0 ** np.floor(np.log2(48.0 / (d * 1.25))))
         for t, d in _DMAX.items()}

_CACHE = {}


def _build(use_btr, use_gb):
    nc = bacc.Bacc("TRN2", target_bir_lowering=False, debug=False)
    AF = mybir.ActivationFunctionType
    OP = mybir.AluOpType

    obsT_d = nc.dram_tensor("obsT", [RP, BS], BF16, kind="ExternalInput")
    wtr_d = nc.dram_tensor("wtr", [RP, Fd], BF16, kind="ExternalInput")
    wih_d = nc.dram_tensor("wih", [M4, 128, KH * 128], BF16, kind="ExternalInput")
    whh_d = nc.dram_tensor("whh", [H, 4 * H], BF16, kind="ExternalInput")
    whh8_d = nc.dram_tensor("whh8", [H, 4 * H], E4, kind="ExternalInput")
    w1_d = nc.dram_tensor("w1", [H, H2], BF16, kind="ExternalInput")
    w2_d = nc.dram_tensor("w2", [H2, A], BF16, kind="ExternalInput")
    bsum_d = nc.dram_tensor("bsum", [4 * H], F32, kind="ExternalInput")
    b1_d = nc.dram_tensor("b1", [H2], F32, kind="ExternalInput")
    b2_d = nc.dram_tensor("b2", [A], F32, kind="ExternalInput")
    if use_btr:
        btr_d = nc.dram_tensor("btr", [Fd], F32, kind="ExternalInput")
    if use_gb:
        gam_d = nc.dram_tensor("gam", [Fd], F32, kind="ExternalInput")
        bet_d = nc.dram_tensor("bet", [Fd], F32, kind="ExternalInput")
    mu_d = nc.dram_tensor("mu", [T, A, BS], F32, kind="ExternalOutput")

    def bc(ap1d, p=128):
        return bass.AP(tensor=ap1d.tensor, offset=ap1d.offset,
                       ap=[[0, p]] + [list(x) for x in ap1d.ap])

    with tile.TileContext(nc) as tc:
        with (
            tc.tile_pool(name="const", bufs=1) as cst,
            tc.tile_pool(name="state", bufs=1) as st,
        ):
            ident = cst.tile([128, 128], BF16)
            whh_sb = cst.tile([128, KH, 4 * H], BF16)     # 64KB/part
            w1_sb = cst.tile([128, KH, H2], BF16)         # 8KB/part
            w2_sb = cst.tile([128, H2 // 128, A], BF16)
            bsum_sb = cst.tile([128, M4], F32)
            b1_sb = cst.tile([128, H2 // 128], F32)
            b2_t = cst.tile([A, 1], F32)
            eps_t = cst.tile([128, 1], F32)

            xT = st.tile([128, KH, BS], BF16)
            preT = st.tile([128, M4, BS], BF16)           # 16KB/part
            gates = st.tile([128, M4, BS], BF16)          # 16KB/part
            c_st = st.tile([128, KH, BS], BF16)           # 4KB/part
            hT = [st.tile([128, KH, BS], BF16, name=f"hT{i}", tag=f"h{i}")
                  for i in range(2)]
            relu1T = st.tile([128, H2 // 128, BS], BF16)

            wtr_r = wtr_d.ap().rearrange("(ko p) n -> p ko n", p=128)
            obsT_r = obsT_d.ap().rearrange("(ko p) b -> p ko b", p=128)

            # ================= Phase 1: trunk GEMM =================
            with tc.tile_pool(name="ps_trunk", bufs=1, space="PSUM") as pst:
                psx = pst.tile([128, NB, Fd], F32)        # 8KB/part = 4 banks
                with tc.tile_pool(name="wstream", bufs=2) as ws:
                    for kg in range(0, NK, KG):
                        kn = min(KG, NK - kg)
                        wt = ws.tile([128, KG, Fd], BF16, tag="wtr", bufs=6)
                        ot = ws.tile([128, KG, BS], BF16, tag="obsT", bufs=4)
                        nc.sync.dma_start(wt[:, :kn, :], wtr_r[:, kg : kg + kn, :])
                        nc.sync.dma_start(ot[:, :kn, :], obsT_r[:, kg : kg + kn, :])
                        if kg == KG:
                            # small consts after the first trunk chunks queued
                            nc.sync.dma_start(
                                bsum_sb, bsum_d.ap().rearrange("(m p) -> p m", p=128))
                            nc.sync.dma_start(
                                b1_sb, b1_d.ap().rearrange("(m p) -> p m", p=128))
                            nc.sync.dma_start(
                                b2_t, b2_d.ap().rearrange("(a one) -> a one", one=1))
                            nc.vector.memset(eps_t, 1e-5)
                            make_identity(nc, ident)
                        for kk in range(kn):
                            k = kg + kk
                            for b in range(NB):
                                lhsT = ot[:, kk, b * 128 : (b + 1) * 128]
                                for n in range(2):
                                    nc.tensor.matmul(
                                        psx[:, b, n * 512 : (n + 1) * 512],
                                        lhsT,
                                        wt[:, kk, n * 512 : (n + 1) * 512],
                                        start=(k == 0),
                                        stop=(k == NK - 1),
                                    )

                # ============ Phase 2: LayerNorm + tanh ============
                with tc.tile_pool(name="lnwork", bufs=1) as wk:
                    if use_btr:
                        btr_b = wk.tile([128, Fd], F32, tag="btr")
                        nc.sync.dma_start(btr_b, bc(btr_d.ap()))
                    if use_gb:
                        gam_b = wk.tile([128, Fd], F32, tag="gam")
                        bet_b = wk.tile([128, Fd], F32, tag="bet")
                        nc.sync.dma_start(gam_b, bc(gam_d.ap()))
                        nc.sync.dma_start(bet_b, bc(bet_d.ap()))
                    xa = wk.tile([128, NB, Fd], BF16, tag="xa")
                    for b in range(NB):
                        if use_btr:
                            xs = wk.tile([128, Fd], F32, tag="xs", bufs=2)
                            nc.vector.tensor_add(xs, psx[:, b, :], btr_b)
                        else:
                            xs = psx[:, b, :]
                        stats = wk.tile([128, 2, 6], F32, tag="stats")
                        for s in range(2):
                            nc.vector.bn_stats(
                                out=stats[:, s, :], in_=xs[:, s * 512 : (s + 1) * 512])
                        mv = wk.tile([128, 2], F32, tag="mv")
                        nc.vector.bn_aggr(out=mv, in_=stats)
                        rstd = wk.tile([128, 1], F32, tag="rstd", bufs=2)
                        nc.scalar.activation(
                            out=rstd, in_=mv[:, 1:2], func=AF.Sqrt,
                            bias=eps_t, scale=1.0)
                        nc.vector.reciprocal(out=rstd, in_=rstd)
                        xn = wk.tile([128, Fd], F32, tag="xn", bufs=2)
                        rstd_b = rstd.to_broadcast([128, Fd])
                        nc.vector.scalar_tensor_tensor(
                            out=xn, in0=xs, scalar=mv[:, 0:1], in1=rstd_b,
                            op0=OP.subtract, op1=OP.mult)
                        if use_gb:
                            nc.vector.scalar_tensor_tensor(
                                out=xn, in0=xn, scalar=1.0, in1=gam_b,
                                op0=OP.mult, op1=OP.mult)
                            nc.vector.tensor_add(xn, xn, bet_b)
                        nc.scalar.activation(out=xa[:, b, :], in_=xn, func=AF.Tanh)

                    # ===== Phase 3: transpose x -> xT (bf16) =====
                    with tc.tile_pool(name="ps_tr", bufs=4, space="PSUM") as ptr:
                        for b in range(NB):
                            for f in range(KH):
                                pt = ptr.tile([128, 128], BF16, tag="tr")
                                nc.tensor.transpose(
                                    pt, xa[:, b, f * 128 : (f + 1) * 128], ident)
                                nc.scalar.activation(
                                    out=xT[:, f, b * 128 : (b + 1) * 128],
                                    in_=pt, func=AF.Copy)

            # ============ lstm-persistent fp8 weights ============
            with tc.tile_pool(name="lstmw", bufs=1) as lw:
                whh8_sb = lw.tile([128, KH, 4 * H], E4)   # 32KB/part
                whh8_r = whh8_d.ap().rearrange("(ko p) n -> p ko n", p=128)
                whh_r = whh_d.ap().rearrange("(ko p) n -> p ko n", p=128)

                # ===== Phase 4: pre^T = W_ih^T x^T + bsum =====
                with (
                    tc.tile_pool(name="wihstream", bufs=2) as ws2,
                    tc.tile_pool(name="ps_pre", bufs=2, space="PSUM") as ppr,
                ):
                    for m in range(M4):
                        wm = ws2.tile([128, KH, 128], BF16, tag="wih", bufs=4)
                        nc.sync.dma_start(
                            wm, wih_d.ap()[m].rearrange("p (k j) -> p k j", j=128))
                        if m % 4 == 0:
                            k8 = m // 4
                            nc.sync.dma_start(whh_sb[:, k8, :], whh_r[:, k8, :])
                            if k8 == 0:
                                nc.sync.dma_start(
                                    w1_sb,
                                    w1_d.ap().rearrange("(ko p) n -> p ko n", p=128))
                                nc.sync.dma_start(
                                    w2_sb,
                                    w2_d.ap().rearrange("(ko p) n -> p ko n", p=128))
                        elif m % 4 == 2:
                            k8 = m // 4
                            nc.sync.dma_start(whh8_sb[:, k8, :], whh8_r[:, k8, :])
                        ps = ppr.tile([128, BS], F32, tag="pre")
                        for k in range(KH):
                            nc.tensor.matmul(
                                ps, wm[:, k, :], xT[:, k, :],
                                start=(k == 0), stop=(k == KH - 1))
                        nc.vector.tensor_scalar_add(
                            preT[:, m, :], ps, bsum_sb[:, m : m + 1])

                # ============ Phase 5: LSTM steps ============
                # Delta-fp8 recurrence with the MLP head DELAYED one step: the
                # PE order per step is [gate matmuls(t)][head matmuls(t-1)], so
                # the head fills the PE while step t's elementwise tail
                # (gates accum -> acts -> cell -> d8) completes. Cell state and
                # all tail intermediates are bf16 for 2x DVE throughput.
                with (
                    tc.tile_pool(name="acts", bufs=2) as ac,
                    tc.tile_pool(name="ps_g", bufs=2, space="PSUM") as psg,
                    tc.tile_pool(name="ps_m", bufs=1, space="PSUM") as psm,
                    tc.tile_pool(name="ps_w2", bufs=1, space="PSUM") as psw,
                ):
                    # group layout: gate q (i,f,g,o), H-half hh (0,1)
                    # m-tiles of group (q, hh): m = 8q + 4*hh + [0..4)
                    def gate_group_matmuls_bf16(q, hh, h_cur):
                        P = psg.tile([128, 4, BS], F32, tag="g")
                        for mi in range(4):
                            m = 8 * q + 4 * hh + mi
                            for kk in range(KH):
                                k = (kk + m) % KH
                                nc.tensor.matmul(
                                    P[:, mi, :],
                                    whh_sb[:, k, m * 128 : (m + 1) * 128],
                                    h_cur[:, k, :],
                                    start=(kk == 0), stop=(kk == KH - 1))
                        return P

                    def gate_group_matmuls_fp8(q, hh, d8):
                        P = psg.tile([128, 4, BS], F32, tag="g")
                        for mi in range(4):
                            m = 8 * q + 4 * hh + mi
                            for kp in range(KH // 2):
                                k = (kp + m) % (KH // 2)
                                nc.tensor.matmul(
                                    P[:, mi, :],
                                    whh8_sb[:, 2 * k : 2 * k + 2,
                                            m * 128 : (m + 1) * 128],
                                    d8[:, 2 * k : 2 * k + 2, :],
                                    start=(kp == 0), stop=(kp == KH // 2 - 1),
                                    perf_mode=DR)
                        return P

                    def half_cell(hh, si, sf, tg, so, first):
                        j0 = 4 * hh
                        csl = c_st[:, j0 : j0 + 4, :]
                        if first:
                            nc.vector.tensor_mul(csl, si, tg)
                        else:
                            t1 = ac.tile([128, 4, BS], BF16, tag="t1")
                            nc.vector.tensor_mul(t1, si, tg)
                            nc.vector.tensor_mul(csl, csl, sf)
                            nc.vector.tensor_add(csl, csl, t1)
                        tcn = ac.tile([128, 4, BS], BF16, tag="tc")
                        nc.scalar.activation(out=tcn, in_=csl, func=AF.Tanh)
                        nc.vector.tensor_mul(h_new[:, j0 : j0 + 4, :], so, tcn)

                    def half_d8(hh, t_next, h_cur, d8):
                        j0 = 4 * hh
                        dtmp = ac.tile([128, 4, BS], BF16, tag="dt")
                        nc.vector.tensor_sub(
                            dtmp, h_new[:, j0 : j0 + 4, :],
                            h_cur[:, j0 : j0 + 4, :])
                        nc.scalar.activation(
                            out=d8[:, j0 : j0 + 4, :], in_=dtmp, func=AF.Copy,
                            scale=SCALE[t_next])

                    def mlp_head(t, h_src):
                        Pm = psm.tile([128, H2 // 128, BS], F32, tag="m1")
                        for m in range(H2 // 128):
                            for k in range(KH):
                                nc.tensor.matmul(
                                    Pm[:, m, :],
                                    w1_sb[:, k, m * 128 : (m + 1) * 128],
                                    h_src[:, k, :],
                                    start=(k == 0), stop=(k == KH - 1))
                        for m in range(H2 // 128):
                            nc.scalar.activation(
                                out=relu1T[:, m, :], in_=Pm[:, m, :],
                                func=AF.Relu, bias=b1_sb[:, m : m + 1], scale=1.0)
                        ps2 = psw.tile([A, BS], F32, tag="w2")
                        for k2 in range(H2 // 128):
                            nc.tensor.matmul(
                                ps2, w2_sb[:, k2, :], relu1T[:, k2, :],
                                start=(k2 == 0), stop=(k2 == H2 // 128 - 1))
                        stage = ac.tile([A, BS], F32, tag="mu")
                        nc.scalar.activation(
                            out=stage, in_=ps2, func=AF.Tanh, bias=b2_t,
                            scale=1.0)
                        nc.sync.dma_start(mu_d.ap()[t], stage)

                    # ---- step 0: h0 = c0 = 0 -> gates = pre ----
                    h_new = hT[0]
                    for hh in range(2):
                        j0 = 4 * hh
                        si = ac.tile([128, 4, BS], BF16, tag="a0")
                        tg = ac.tile([128, 4, BS], BF16, tag="a2")
                        so = ac.tile([128, 4, BS], BF16, tag="a3")
                        nc.scalar.activation(
                            out=si, in_=preT[:, j0 : j0 + 4, :], func=AF.Sigmoid)
                        nc.scalar.activation(
                            out=tg, in_=preT[:, 16 + j0 : 16 + j0 + 4, :],
                            func=AF.Tanh)
                        nc.scalar.activation(
                            out=so, in_=preT[:, 24 + j0 : 24 + j0 + 4, :],
                            func=AF.Sigmoid)
                        half_cell(hh, si, None, tg, so, first=True)
                    d8_cur = None

                    # ---- steps 1..15 (head for t-1 emitted inside step t) ----
                    for t in range(1, T):
                        h_cur = hT[(t + 1) % 2]
                        h_new = hT[t % 2]
                        is_refresh = t in REFRESH
                        if not is_refresh:
                            dsc = 1.0 / (SW * SCALE[t])
                        need_d8 = (t + 1 < T) and ((t + 1) not in REFRESH)
                        d8_next = None
                        if need_d8:
                            d8_next = lw.tile([128, KH, BS], E4, tag="d8", bufs=2)
                        for hh in range(2):
                            j0 = 4 * hh
                            acts = {}
                            for q in range(4):
                                g0 = 8 * q + j0
                                gsl = gates[:, g0 : g0 + 4, :]
                                if is_refresh:
                                    P = gate_group_matmuls_bf16(q, hh, h_cur)
                                    nc.vector.tensor_add(
                                        gsl, P, preT[:, g0 : g0 + 4, :])
                                else:
                                    P = gate_group_matmuls_fp8(q, hh, d8_cur)
                                    nc.vector.scalar_tensor_tensor(
                                        out=gsl, in0=P, scalar=dsc, in1=gsl,
                                        op0=OP.mult, op1=OP.add)
                                out_a = ac.tile([128, 4, BS], BF16, tag=f"a{q}")
                                nc.scalar.activation(
                                    out=out_a, in_=gsl,
                                    func=AF.Tanh if q == 2 else AF.Sigmoid)
                                acts[q] = out_a
                            half_cell(hh, acts[0], acts[1], acts[2], acts[3],
                                      first=False)
                            if need_d8:
                                half_d8(hh, t + 1, h_cur, d8_next)
                        mlp_head(t - 1, h_cur)
                        d8_cur = d8_next
                    mlp_head(T - 1, hT[(T - 1) % 2])

    nc.compile()
    return nc


def kernel(**inputs):
    obs = np.asarray(inputs["obs"], np.float32)
    W_trunk = np.asarray(inputs["W_trunk"], np.float32)
    b_trunk = np.asarray(inputs["b_trunk"], np.float32)
    gamma = np.asarray(inputs["gamma"], np.float32)
    beta = np.asarray(inputs["beta"], np.float32)
    W_ih = np.asarray(inputs["W_ih"], np.float32)
    b_ih = np.asarray(inputs["b_ih"], np.float32)
    W_hh = np.asarray(inputs["W_hh"], np.float32)
    b_hh = np.asarray(inputs["b_hh"], np.float32)
    W1 = np.asarray(inputs["W1"], np.float32)
    b1 = np.asarray(inputs["b1"], np.float32)
    W2 = np.asarray(inputs["W2"], np.float32)
    b2 = np.asarray(inputs["b2"], np.float32)
    num_actions = int(np.asarray(inputs["num_actions"]))
    assert num_actions == T, f"kernel hardcodes T={T}, got {num_actions}"
    assert obs.shape == (B, R)

    use_btr = bool(b_trunk.any())
    use_gb = bool((gamma != 1).any() or beta.any())
    key = (use_btr, use_gb)
    if key not in _CACHE:
        _CACHE[key] = _build(use_btr, use_gb)
    nc = _CACHE[key]

    wtr = np.zeros((RP, Fd), BF)
    wtr[:R] = W_trunk.astype(BF)
    wih = np.ascontiguousarray(
        W_ih.astype(BF).reshape(KH, 128, M4, 128).transpose(2, 1, 0, 3)
    ).reshape(M4, 128, KH * 128)
    whh = W_hh.astype(BF)
    whh8 = (W_hh * SW).astype(E4NP)
    w1 = W1.astype(BF)
    w2 = W2.astype(BF)
    bsum = (b_ih + b_hh).astype(np.float32)

    in_maps = []
    for i in range(NC_):
        sh = obs[i * BS : (i + 1) * BS]           # [256, R]
        obsT = np.zeros((RP, BS), BF)
        obsT[:R] = np.ascontiguousarray(sh.T).astype(BF)
        m = {
            "obsT": obsT, "wtr": wtr, "wih": wih, "whh": whh, "whh8": whh8,
            "w1": w1, "w2": w2, "bsum": bsum, "b1": b1, "b2": b2,
        }
        if use_btr:
            m["btr"] = b_trunk
        if use_gb:
            m["gam"] = gamma
            m["bet"] = beta
        in_maps.append(m)

    res = bass_utils.run_bass_kernel_spmd(
        nc, in_maps, core_ids=list(range(NC_)),
        trace=bool(int(__import__("os").environ.get("KTRACE", "0"))),
    )
    _CACHE["last_result"] = res
    # per-core mu is [T, A, BS]; assemble to [B, T, A]
    full = np.concatenate([res.results[i]["mu"] for i in range(NC_)], axis=2)
    out = np.ascontiguousarray(np.transpose(full, (2, 0, 1)))
    return out


# revision 14
# speedup vs baseline: 1.0681x; 1.0681x over previous
"""Trainium2 Bass kernel for nn_LSTMActor: trunk GEMM -> LayerNorm -> Tanh ->
LSTM (16 steps, constant input) -> MLP head -> tanh.

Sharding: data-parallel over batch B=2048 across 8 cores (256 rows each);
weights replicated. Transposed layout throughout (feature dim on partitions).

Perf scheme vs plain bf16:
- LSTM recurrent matmul uses delta-sigma fp8: on "delta" steps the gates are
  updated incrementally, gates += W8^T @ fp8(h_t - h_{t-1}), with the fp8
  DoubleRow perf mode (2 k-subtiles per instruction = 2x matmul throughput).
  Quantization error is relative to |dh| (small and shrinking), and the fixed
  W8 quantization error telescopes across steps; full-bf16 "refresh" steps at
  t in {1,5} bound the residual W-side error window. Measured end-to-end
  error ~9.8e-3 vs the 2e-2 budget.
- The MLP head for step t-1 is emitted after step t's gate matmuls, filling
  the PE while the elementwise tail (gates accum -> acts -> cell -> d8) runs.
- Elementwise work runs in fused multi-tile ops (gate groups of 4 m-tiles,
  H-halves for the cell, bf16 intermediates for 2x DVE rate).
- W2 head runs transposed (stationary = W2, 6 output partitions) and mu is
  DMA'd out per step as [T, A, BS]; the host reassembles [B, T, A].
"""

import numpy as np
import ml_dtypes

import concourse.bass as bass
import concourse.tile as tile
from concourse import mybir, bacc
from concourse import bass_utils
from concourse.masks import make_identity

BF = ml_dtypes.bfloat16
E4NP = ml_dtypes.float8_e4m3
F32 = mybir.dt.float32
BF16 = mybir.dt.bfloat16
E4 = mybir.dt.float8e4
DR = mybir.MatmulPerfMode.DoubleRow
AluOp = None  # set in _build

B, R, Fd, H, A, T = 2048, 39200, 1024, 1024, 6, 16
NC_ = 8
BS = B // NC_          # 256 rows per core
NB = BS // 128         # 2 b-tiles per core
KT = 128
RP = ((R + KT - 1) // KT) * KT   # 39296
NK = RP // KT          # 307 K-tiles for trunk
KH = H // 128          # 8 K-tiles for H-dim GEMMs
M4 = 4 * H // 128      # 32 M-tiles of gates
H2 = H // 2            # 512
KG = 2                 # trunk K-tiles per DMA batch

REFRESH = (1, 5)
SW = 64.0              # W_hh fp8 scale
# measured max|h_t - h_{t-1}| of the bf16 trajectory, x1.25 safety
_DMAX = {2: 0.247, 3: 0.163, 4: 0.136, 5: 0.113, 6: 0.098, 7: 0.084,
         8: 0.072, 9: 0.057, 10: 0.049, 11: 0.0445, 12: 0.0435, 13: 0.041,
         14: 0.038, 15: 0.035}
SCALE = {t: float(2.0 ** np.floor(np.log2(48.0 / (d * 1.25))))
         for t, d in _DMAX.items()}

_CACHE = {}


def _build(use_btr, use_gb):
    nc = bacc.Bacc("TRN2", target_bir_lowering=False, debug=False)
    AF = mybir.ActivationFunctionType
    OP = mybir.AluOpType

    obsT_d = nc.dram_tensor("obsT", [RP, BS], BF16, kind="ExternalInput")
    wtr_d = nc.dram_tensor("wtr", [RP, Fd], BF16, kind="ExternalInput")
    wih_d = nc.dram_tensor("wih", [M4, 128, KH * 128], BF16, kind="ExternalInput")
    whh_d = nc.dram_tensor("whh", [H, 4 * H], BF16, kind="ExternalInput")
    whh8_d = nc.dram_tensor("whh8", [H, 4 * H], E4, kind="ExternalInput")
    w1_d = nc.dram_tensor("w1", [H, H2], BF16, kind="ExternalInput")
    w2_d = nc.dram_tensor("w2", [H2, A], BF16, kind="ExternalInput")
    bsum_d = nc.dram_tensor("bsum", [4 * H], F32, kind="ExternalInput")
    b1_d = nc.dram_tensor("b1", [H2], F32, kind="ExternalInput")
    b2_d = nc.dram_tensor("b2", [A], F32, kind="ExternalInput")
    if use_btr:
        btr_d = nc.dram_tensor("btr", [Fd], F32, kind="ExternalInput")
    if use_gb:
        gam_d = nc.dram_tensor("gam", [Fd], F32, kind="ExternalInput")
        bet_d = nc.dram_tensor("bet", [Fd], F32, kind="ExternalInput")
    mu_d = nc.dram_tensor("mu", [T, A, BS], F32, kind="ExternalOutput")

    def bc(ap1d, p=128):
        return bass.AP(tensor=ap1d.tensor, offset=ap1d.offset,
                       ap=[[0, p]] + [list(x) for x in ap1d.ap])

    with tile.TileContext(nc) as tc:
        with (
            tc.tile_pool(name="const", bufs=1) as cst,
            tc.tile_pool(name="state", bufs=1) as st,
        ):
            ident = cst.tile([128, 128], BF16)
            whh_sb = cst.tile([128, KH, 4 * H], BF16)     # 64KB/part
            w1_sb = cst.tile([128, KH, H2], BF16)         # 8KB/part
            w2_sb = cst.tile([128, H2 // 128, A], BF16)
            bsum_sb = cst.tile([128, M4], F32)
            b1_sb = cst.tile([128, H2 // 128], F32)
            b2_t = cst.tile([A, 1], F32)
            eps_t = cst.tile([128, 1], F32)

            xT = st.tile([128, KH, BS], BF16)
            preT = st.tile([128, M4, BS], BF16)           # 16KB/part
            gates = st.tile([128, M4, BS], BF16)          # 16KB/part
            c_st = st.tile([128, KH, BS], BF16)           # 4KB/part
            hT = [st.tile([128, KH, BS], BF16, name=f"hT{i}", tag=f"h{i}")
                  for i in range(2)]
            relu1T = st.tile([128, H2 // 128, BS], BF16)

            wtr_r = wtr_d.ap().rearrange("(ko p) n -> p ko n", p=128)
            obsT_r = obsT_d.ap().rearrange("(ko p) b -> p ko b", p=128)

            # ================= Phase 1: trunk GEMM =================
            with tc.tile_pool(name="ps_trunk", bufs=1, space="PSUM") as pst:
                psx = pst.tile([128, NB, Fd], F32)        # 8KB/part = 4 banks
                with tc.tile_pool(name="wstream", bufs=2) as ws:
                    for kg in range(0, NK, KG):
                        kn = min(KG, NK - kg)
                        wt = ws.tile([128, KG, Fd], BF16, tag="wtr", bufs=6)
                        ot = ws.tile([128, KG, BS], BF16, tag="obsT", bufs=4)
                        nc.sync.dma_start(wt[:, :kn, :], wtr_r[:, kg : kg + kn, :])
                        nc.sync.dma_start(ot[:, :kn, :], obsT_r[:, kg : kg + kn, :])
                        if kg == KG:
                            # small consts after the first trunk chunks queued
                            nc.sync.dma_start(
                                bsum_sb, bsum_d.ap().rearrange("(m p) -> p m", p=128))
                            nc.sync.dma_start(
                                b1_sb, b1_d.ap().rearrange("(m p) -> p m", p=128))
                            nc.sync.dma_start(
                                b2_t, b2_d.ap().rearrange("(a one) -> a one", one=1))
                            nc.vector.memset(eps_t, 1e-5)
                            make_identity(nc, ident)
                        for kk in range(kn):
                            k = kg + kk
                            for b in range(NB):
                                lhsT = ot[:, kk, b * 128 : (b + 1) * 128]
                                for n in range(2):
                                    nc.tensor.matmul(
                                        psx[:, b, n * 512 : (n + 1) * 512],
                                        lhsT,
                                        wt[:, kk, n * 512 : (n + 1) * 512],
                                        start=(k == 0),
                                        stop=(k == NK - 1),
                                    )

                # ============ Phase 2: LayerNorm + tanh ============
                with tc.tile_pool(name="lnwork", bufs=1) as wk:
                    if use_btr:
                        btr_b = wk.tile([128, Fd], F32, tag="btr")
                        nc.sync.dma_start(btr_b, bc(btr_d.ap()))
                    if use_gb:
                        gam_b = wk.tile([128, Fd], F32, tag="gam")
                        bet_b = wk.tile([128, Fd], F32, tag="bet")
                        nc.sync.dma_start(gam_b, bc(gam_d.ap()))
                        nc.sync.dma_start(bet_b, bc(bet_d.ap()))
                    xa = wk.tile([128, NB, Fd], BF16, tag="xa")
                    for b in range(NB):
                        if use_btr:
                            xs = wk.tile([128, Fd], F32, tag="xs", bufs=2)
                            nc.vector.tensor_add(xs, psx[:, b, :], btr_b)
                        else:
                            xs = psx[:, b, :]
                        stats = wk.tile([128, 2, 6], F32, tag="stats")
                        for s in range(2):
                            nc.vector.bn_stats(
                                out=stats[:, s, :], in_=xs[:, s * 512 : (s + 1) * 512])
                        mv = wk.tile([128, 2], F32, tag="mv")
                        nc.vector.bn_aggr(out=mv, in_=stats)
                        rstd = wk.tile([128, 1], F32, tag="rstd", bufs=2)
                        nc.scalar.activation(
                            out=rstd, in_=mv[:, 1:2], func=AF.Sqrt,
                            bias=eps_t, scale=1.0)
                        nc.vector.reciprocal(out=rstd, in_=rstd)
                        xn = wk.tile([128, Fd], F32, tag="xn", bufs=2)
                        rstd_b = rstd.to_broadcast([128, Fd])
                        nc.vector.scalar_tensor_tensor(
                            out=xn, in0=xs, scalar=mv[:, 0:1], in1=rstd_b,
                            op0=OP.subtract, op1=OP.mult)
                        if use_gb:
                            nc.vector.scalar_tensor_tensor(
                                out=xn, in0=xn, scalar=1.0, in1=gam_b,
                                op0=OP.mult, op1=OP.mult)
                            nc.vector.tensor_add(xn, xn, bet_b)
                        nc.scalar.activation(out=xa[:, b, :], in_=xn, func=AF.Tanh)

                    # ===== Phase 3: transpose x -> xT (bf16) =====
                    with tc.tile_pool(name="ps_tr", bufs=4, space="PSUM") as ptr:
                        for b in range(NB):
                            for f in range(KH):
                                pt = ptr.tile([128, 128], BF16, tag="tr")
                                nc.tensor.transpose(
                                    pt, xa[:, b, f * 128 : (f + 1) * 128], ident)
                                nc.scalar.activation(
                                    out=xT[:, f, b * 128 : (b + 1) * 128],
                                    in_=pt, func=AF.Copy)

            # ============ lstm-persistent fp8 weights ============
            with tc.tile_pool(name="lstmw", bufs=1) as lw:
                whh8_sb = lw.tile([128, KH, 4 * H], E4)   # 32KB/part
                whh8_r = whh8_d.ap().rearrange("(ko p) n -> p ko n", p=128)
                whh_r = whh_d.ap().rearrange("(ko p) n -> p ko n", p=128)

                # ===== Phase 4: pre^T = W_ih^T x^T + bsum =====
                with (
                    tc.tile_pool(name="wihstream", bufs=2) as ws2,
                    tc.tile_pool(name="ps_pre", bufs=2, space="PSUM") as ppr,
                ):
                    for m in range(M4):
                        wm = ws2.tile([128, KH, 128], BF16, tag="wih", bufs=4)
                        nc.sync.dma_start(
                            wm, wih_d.ap()[m].rearrange("p (k j) -> p k j", j=128))
                        if m % 4 == 0:
                            k8 = m // 4
                            nc.sync.dma_start(whh_sb[:, k8, :], whh_r[:, k8, :])
                            if k8 == 0:
                                nc.sync.dma_start(
                                    w1_sb,
                                    w1_d.ap().rearrange("(ko p) n -> p ko n", p=128))
                                nc.sync.dma_start(
                                    w2_sb,
                                    w2_d.ap().rearrange("(ko p) n -> p ko n", p=128))
                        elif m % 4 == 2:
                            k8 = m // 4
                            nc.sync.dma_start(whh8_sb[:, k8, :], whh8_r[:, k8, :])
                        ps = ppr.tile([128, BS], F32, tag="pre")
                        for k in range(KH):
                            nc.tensor.matmul(
                                ps, wm[:, k, :], xT[:, k, :],
                                start=(k == 0), stop=(k == KH - 1))
                        nc.vector.tensor_scalar_add(
                            preT[:, m, :], ps, bsum_sb[:, m : m + 1])

                # ============ Phase 5: LSTM steps ============
                # Delta-fp8 recurrence with the MLP head DELAYED one step: the
                # PE order per step is [gate matmuls(t)][head matmuls(t-1)], so
                # the head fills the PE while step t's elementwise tail
                # (gates accum -> acts -> cell -> d8) completes. Cell state and
                # all tail intermediates are bf16 for 2x DVE throughput.
                with (
                    tc.tile_pool(name="acts", bufs=2) as ac,
                    tc.tile_pool(name="ps_g", bufs=2, space="PSUM") as psg,
                    tc.tile_pool(name="ps_m", bufs=1, space="PSUM") as psm,
                    tc.tile_pool(name="ps_w2", bufs=1, space="PSUM") as psw,
                ):
                    # group layout: gate q (i,f,g,o), H-half hh (0,1)
                    # m-tiles of group (q, hh): m = 8q + 4*hh + [0..4)
                    def gate_group_matmuls_bf16(q, hh, h_cur):
                        P = psg.tile([128, 4, BS], F32, tag="g")
                        for mi in range(4):
                            m = 8 * q + 4 * hh + mi
                            for kk in range(KH):
                                k = (kk + m) % KH
                                nc.tensor.matmul(
                                    P[:, mi, :],
                                    whh_sb[:, k, m * 128 : (m + 1) * 128],
                                    h_cur[:, k, :],
                                    start=(kk == 0), stop=(kk == KH - 1))
                        return P

                    def gate_group_matmuls_fp8(q, hh, d8):
                        P = psg.tile([128, 4, BS], F32, tag="g")
                        for mi in range(4):
                            m = 8 * q + 4 * hh + mi
                            for kp in range(KH // 2):
                                k = (kp + m) % (KH // 2)
                                nc.tensor.matmul(
                                    P[:, mi, :],
                                    whh8_sb[:, 2 * k : 2 * k + 2,
                                            m * 128 : (m + 1) * 128],
                                    d8[:, 2 * k : 2 * k + 2, :],
                                    start=(kp == 0), stop=(kp == KH // 2 - 1),
                                    perf_mode=DR)
                        return P

                    def half_cell(hh, si, sf, tg, so, first):
                        j0 = 4 * hh
                        csl = c_st[:, j0 : j0 + 4, :]
                        if first:
                            nc.vector.tensor_mul(csl, si, tg)
                        else:
                            t1 = ac.tile([128, 4, BS], BF16, tag="t1")
                            nc.vector.tensor_mul(t1, si, tg)
                            nc.vector.tensor_mul(csl, csl, sf)
                            nc.vector.tensor_add(csl, csl, t1)
                        tcn = ac.tile([128, 4, BS], BF16, tag="tc")
                        nc.scalar.activation(out=tcn, in_=csl, func=AF.Tanh)
                        nc.vector.tensor_mul(h_new[:, j0 : j0 + 4, :], so, tcn)

                    def half_d8(hh, t_next, h_cur, d8):
                        j0 = 4 * hh
                        dtmp = ac.tile([128, 4, BS], BF16, tag="dt")
                        nc.vector.tensor_sub(
                            dtmp, h_new[:, j0 : j0 + 4, :],
                            h_cur[:, j0 : j0 + 4, :])
                        nc.scalar.activation(
                            out=d8[:, j0 : j0 + 4, :], in_=dtmp, func=AF.Copy,
                            scale=SCALE[t_next])

                    def mlp_head(t, h_src):
                        Pm = psm.tile([128, H2 // 128, BS], F32, tag="m1")
                        for m in range(H2 // 128):
                            for k in range(KH):
                                nc.tensor.matmul(
                                    Pm[:, m, :],
                                    w1_sb[:, k, m * 128 : (m + 1) * 128],
                                    h_src[:, k, :],
                                    start=(k == 0), stop=(k == KH - 1))
                        for m in range(H2 // 128):
                            nc.scalar.activation(
                                out=relu1T[:, m, :], in_=Pm[:, m, :],
                                func=AF.Relu, bias=b1_sb[:, m : m + 1], scale=1.0)
                        ps2 = psw.tile([A, BS], F32, tag="w2")
                        for k2 in range(H2 // 128):
                            nc.tensor.matmul(
                                ps2, w2_sb[:, k2, :], relu1T[:, k2, :],
                                start=(k2 == 0), stop=(k2 == H2 // 128 - 1))
                        stage = ac.tile([A, BS], F32, tag="mu")
                        nc.scalar.activation(
                            out=stage, in_=ps2, func=AF.Tanh, bias=b2_t,
                            scale=1.0)
                        nc.sync.dma_start(mu_d.ap()[t], stage)

                    # ---- step 0: h0 = c0 = 0 -> gates = pre ----
                    h_new = hT[0]
                    for hh in range(2):
                        j0 = 4 * hh
                        si = ac.tile([128, 4, BS], BF16, tag="a0")
                        tg = ac.tile([128, 4, BS], BF16, tag="a2")
                        so = ac.tile([128, 4, BS], BF16, tag="a3")
                        nc.scalar.activation(
                            out=si, in_=preT[:, j0 : j0 + 4, :], func=AF.Sigmoid)
                        nc.scalar.activation(
                            out=tg, in_=preT[:, 16 + j0 : 16 + j0 + 4, :],
                            func=AF.Tanh)
                        nc.scalar.activation(
                            out=so, in_=preT[:, 24 + j0 : 24 + j0 + 4, :],
                            func=AF.Sigmoid)
                        half_cell(hh, si, None, tg, so, first=True)
                    d8_cur = None

                    # ---- steps 1..15 (head for t-1 emitted inside step t) ----
                    for t in range(1, T):
                        h_cur = hT[(t + 1) % 2]
                        h_new = hT[t % 2]
                        is_refresh = t in REFRESH
                        if not is_refresh:
                            dsc = 1.0 / (SW * SCALE[t])
                        need_d8 = (t + 1 < T) and ((t + 1) not in REFRESH)
                        d8_next = None
                        if need_d8:
                            d8_next = lw.tile([128, KH, BS], E4, tag="d8", bufs=2)
                        for hh in range(2):
                            j0 = 4 * hh
                            acts = {}
                            for q in range(4):
                                g0 = 8 * q + j0
                                gsl = gates[:, g0 : g0 + 4, :]
                                if is_refresh:
                                    P = gate_group_matmuls_bf16(q, hh, h_cur)
                                    nc.vector.tensor_add(
                                        gsl, P, preT[:, g0 : g0 + 4, :])
                                else:
                                    P = gate_group_matmuls_fp8(q, hh, d8_cur)
                                    nc.vector.scalar_tensor_tensor(
                                        out=gsl, in0=P, scalar=dsc, in1=gsl,
                                        op0=OP.mult, op1=OP.add)
                                out_a = ac.tile([128, 4, BS], BF16, tag=f"a{q}")
                                nc.scalar.activation(
                                    out=out_a, in_=gsl,
                                    func=AF.Tanh if q == 2 else AF.Sigmoid)
                                acts[q] = out_a
                            half_cell(hh, acts[0], acts[1], acts[2], acts[3],
                                      first=False)
                            if need_d8:
                                half_d8(hh, t + 1, h_cur, d8_next)
                        mlp_head(t - 1, h_cur)
                        d8_cur = d8_next
                    mlp_head(T - 1, hT[(T - 1) % 2])

    nc.compile()
    return nc


def kernel(**inputs):
    obs = np.asarray(inputs["obs"], np.float32)
    W_trunk = np.asarray(inputs["W_trunk"], np.float32)
    b_trunk = np.asarray(inputs["b_trunk"], np.float32)
    gamma = np.asarray(inputs["gamma"], np.float32)
    beta = np.asarray(inputs["beta"], np.float32)
    W_ih = np.asarray(inputs["W_ih"], np.float32)
    b_ih = np.asarray(inputs["b_ih"], np.float32)
    W_hh = np.asarray(inputs["W_hh"], np.float32)
    b_hh = np.asarray(inputs["b_hh"], np.float32)
    W1 = np.asarray(inputs["W1"], np.float32)
    b1 = np.asarray(inputs["b1"], np.float32)
    W2 = np.asarray(inputs["W2"], np.float32)
    b2 = np.asarray(inputs["b2"], np.float32)
    num_actions = int(np.asarray(inputs["num_actions"]))
    assert num_actions == T, f"kernel hardcodes T={T}, got {num_actions}"
    assert obs.shape == (B, R)

    use_btr = bool(b_trunk.any())
    use_gb = bool((gamma != 1).any() or beta.any())
    key = (use_btr, use_gb)
    if key not in _CACHE:
        _CACHE[key] = _build(use_btr, use_gb)
    nc = _CACHE[key]

    wtr = np.zeros((RP, Fd), BF)
    wtr[:R] = W_trunk.astype(BF)
    wih = np.ascontiguousarray(
        W_ih.astype(BF).reshape(KH, 128, M4, 128).transpose(2, 1, 0, 3)
    ).reshape(M4, 128, KH * 128)
    whh = W_hh.astype(BF)
    whh8 = (W_hh * SW).astype(E4NP)
    w1 = W1.astype(BF)
    w2 = W2.astype(BF)
    bsum = (b_ih + b_hh).astype(np.float32)

    in_maps = []
    for i in range(NC_):
        sh = obs[i * BS : (i + 1) * BS]           # [256, R]
        obsT = np.zeros((RP, BS), BF)
        obsT[:R] = np.ascontiguousarray(sh.T).astype(BF)
        m = {
            "obsT": obsT, "wtr": wtr, "wih": wih, "whh": whh, "whh8": whh8,
            "w1": w1, "w2": w2, "bsum": bsum, "b1": b1, "b2": b2,
        }
        if use_btr:
            m["btr"] = b_trunk
        if use_gb:
            m["gam"] = gamma
            m["bet"] = beta
        in_maps.append(m)

    res = bass_utils.run_bass_kernel_spmd(
        nc, in_maps, core_ids=list(range(NC_)),
        trace=bool(int(__import__("os").environ.get("KTRACE", "0"))),
    )
    _CACHE["last_result"] = res
    # per-core mu is [T, A, BS]; assemble to [B, T, A]
    full = np.concatenate([res.results[i]["mu"] for i in range(NC_)], axis=2)
    out = np.ascontiguousarray(np.transpose(full, (2, 0, 1)))
    return out


# revision 15
# speedup vs baseline: 1.0950x; 1.0253x over previous
"""Trainium2 Bass kernel for nn_LSTMActor: trunk GEMM -> LayerNorm -> Tanh ->
LSTM (16 steps, constant input) -> MLP head -> tanh.

Sharding: data-parallel over batch B=2048 across 8 cores (256 rows each);
weights replicated. Transposed layout throughout (feature dim on partitions).

Perf scheme vs plain bf16:
- LSTM recurrent matmul uses delta-sigma fp8: on "delta" steps the gates are
  updated incrementally, gates += W8^T @ fp8(h_t - h_{t-1}), with the fp8
  DoubleRow perf mode (2 k-subtiles per instruction = 2x matmul throughput).
  Quantization error is relative to |dh| (small and shrinking), and the fixed
  W8 quantization error telescopes across steps; full-bf16 "refresh" steps at
  t in {1,5} bound the residual W-side error window. Measured end-to-end
  error ~9.8e-3 vs the 2e-2 budget.
- The MLP head for step t-1 is emitted after step t's gate matmuls, filling
  the PE while the elementwise tail (gates accum -> acts -> cell -> d8) runs.
- Elementwise work runs in fused multi-tile ops (gate groups of 4 m-tiles,
  H-halves for the cell, bf16 intermediates for 2x DVE rate).
- W2 head runs transposed (stationary = W2, 6 output partitions) and mu is
  DMA'd out per step as [T, A, BS]; the host reassembles [B, T, A].
"""

import numpy as np
import ml_dtypes

import concourse.bass as bass
import concourse.tile as tile
from concourse import mybir, bacc
from concourse import bass_utils
from concourse.masks import make_identity

BF = ml_dtypes.bfloat16
E4NP = ml_dtypes.float8_e4m3
F32 = mybir.dt.float32
BF16 = mybir.dt.bfloat16
E4 = mybir.dt.float8e4
DR = mybir.MatmulPerfMode.DoubleRow
AluOp = None  # set in _build

B, R, Fd, H, A, T = 2048, 39200, 1024, 1024, 6, 16
NC_ = 8
BS = B // NC_          # 256 rows per core
NB = BS // 128         # 2 b-tiles per core
KT = 128
RP = ((R + KT - 1) // KT) * KT   # 39296
NK = RP // KT          # 307 K-tiles for trunk
KH = H // 128          # 8 K-tiles for H-dim GEMMs
M4 = 4 * H // 128      # 32 M-tiles of gates
H2 = H // 2            # 512
KG = 2                 # trunk K-tiles per DMA batch

REFRESH = (1, 5)
SW = 64.0              # W_hh fp8 scale
# measured max|h_t - h_{t-1}| of the bf16 trajectory, x1.25 safety
_DMAX = {2: 0.247, 3: 0.163, 4: 0.136, 5: 0.113, 6: 0.098, 7: 0.084,
         8: 0.072, 9: 0.057, 10: 0.049, 11: 0.0445, 12: 0.0435, 13: 0.041,
         14: 0.038, 15: 0.035}
SCALE = {t: float(2.0 ** np.floor(np.log2(48.0 / (d * 1.25))))
         for t, d in _DMAX.items()}

_CACHE = {}


def _build(use_btr, use_gb):
    nc = bacc.Bacc("TRN2", target_bir_lowering=False, debug=False)
    AF = mybir.ActivationFunctionType
    OP = mybir.AluOpType

    obsT_d = nc.dram_tensor("obsT", [RP, BS], BF16, kind="ExternalInput")
    wtr_d = nc.dram_tensor("wtr", [RP, Fd], BF16, kind="ExternalInput")
    wih_d = nc.dram_tensor("wih", [M4, 128, KH * 128], BF16, kind="ExternalInput")
    whh_d = nc.dram_tensor("whh", [H, 4 * H], BF16, kind="ExternalInput")
    whh8_d = nc.dram_tensor("whh8", [H, 4 * H], E4, kind="ExternalInput")
    w1_d = nc.dram_tensor("w1", [H, H2], BF16, kind="ExternalInput")
    w2_d = nc.dram_tensor("w2", [H2, A], BF16, kind="ExternalInput")
    bsum_d = nc.dram_tensor("bsum", [4 * H], F32, kind="ExternalInput")
    b1_d = nc.dram_tensor("b1", [H2], F32, kind="ExternalInput")
    b2_d = nc.dram_tensor("b2", [A], F32, kind="ExternalInput")
    if use_btr:
        btr_d = nc.dram_tensor("btr", [Fd], F32, kind="ExternalInput")
    if use_gb:
        gam_d = nc.dram_tensor("gam", [Fd], F32, kind="ExternalInput")
        bet_d = nc.dram_tensor("bet", [Fd], F32, kind="ExternalInput")
    mu_d = nc.dram_tensor("mu", [T, A, BS], F32, kind="ExternalOutput")

    def bc(ap1d, p=128):
        return bass.AP(tensor=ap1d.tensor, offset=ap1d.offset,
                       ap=[[0, p]] + [list(x) for x in ap1d.ap])

    with tile.TileContext(nc) as tc:
        with (
            tc.tile_pool(name="const", bufs=1) as cst,
            tc.tile_pool(name="state", bufs=1) as st,
        ):
            ident = cst.tile([128, 128], BF16)
            whh_sb = cst.tile([128, KH, 4 * H], BF16)     # 64KB/part
            w1_sb = cst.tile([128, KH, H2], BF16)         # 8KB/part
            w2_sb = cst.tile([128, H2 // 128, A], BF16)
            bsum_sb = cst.tile([128, M4], F32)
            b1_sb = cst.tile([128, H2 // 128], F32)
            b2_t = cst.tile([A, 1], F32)
            eps_t = cst.tile([128, 1], F32)

            xT = st.tile([128, KH, BS], BF16)
            preT = st.tile([128, M4, BS], BF16)           # 16KB/part
            gates = st.tile([128, M4, BS], BF16)          # 16KB/part
            c_st = st.tile([128, KH, BS], BF16)           # 4KB/part
            hT = [st.tile([128, KH, BS], BF16, name=f"hT{i}", tag=f"h{i}")
                  for i in range(2)]
            relu1T = st.tile([128, H2 // 128, BS], BF16)

            wtr_r = wtr_d.ap().rearrange("(ko p) n -> p ko n", p=128)
            obsT_r = obsT_d.ap().rearrange("(ko p) b -> p ko b", p=128)

            # ================= Phase 1: trunk GEMM =================
            with tc.tile_pool(name="ps_trunk", bufs=1, space="PSUM") as pst:
                psx = pst.tile([128, NB, Fd], F32)        # 8KB/part = 4 banks
                with tc.tile_pool(name="wstream", bufs=2) as ws:
                    for kg in range(0, NK, KG):
                        kn = min(KG, NK - kg)
                        wt = ws.tile([128, KG, Fd], BF16, tag="wtr", bufs=6)
                        ot = ws.tile([128, KG, BS], BF16, tag="obsT", bufs=4)
                        nc.sync.dma_start(wt[:, :kn, :], wtr_r[:, kg : kg + kn, :])
                        nc.scalar.dma_start(ot[:, :kn, :], obsT_r[:, kg : kg + kn, :])
                        if kg == KG:
                            # small consts after the first trunk chunks queued
                            nc.sync.dma_start(
                                bsum_sb, bsum_d.ap().rearrange("(m p) -> p m", p=128))
                            nc.sync.dma_start(
                                b1_sb, b1_d.ap().rearrange("(m p) -> p m", p=128))
                            nc.sync.dma_start(
                                b2_t, b2_d.ap().rearrange("(a one) -> a one", one=1))
                            nc.vector.memset(eps_t, 1e-5)
                            make_identity(nc, ident)
                        for kk in range(kn):
                            k = kg + kk
                            for b in range(NB):
                                lhsT = ot[:, kk, b * 128 : (b + 1) * 128]
                                for n in range(2):
                                    nc.tensor.matmul(
                                        psx[:, b, n * 512 : (n + 1) * 512],
                                        lhsT,
                                        wt[:, kk, n * 512 : (n + 1) * 512],
                                        start=(k == 0),
                                        stop=(k == NK - 1),
                                    )

                # ============ Phase 2: LayerNorm + tanh ============
                with tc.tile_pool(name="lnwork", bufs=1) as wk:
                    if use_btr:
                        btr_b = wk.tile([128, Fd], F32, tag="btr")
                        nc.sync.dma_start(btr_b, bc(btr_d.ap()))
                    if use_gb:
                        gam_b = wk.tile([128, Fd], F32, tag="gam")
                        bet_b = wk.tile([128, Fd], F32, tag="bet")
                        nc.sync.dma_start(gam_b, bc(gam_d.ap()))
                        nc.sync.dma_start(bet_b, bc(bet_d.ap()))
                    xa = wk.tile([128, NB, Fd], BF16, tag="xa")
                    for b in range(NB):
                        if use_btr:
                            xs = wk.tile([128, Fd], F32, tag="xs", bufs=2)
                            nc.vector.tensor_add(xs, psx[:, b, :], btr_b)
                        else:
                            xs = psx[:, b, :]
                        stats = wk.tile([128, 2, 6], F32, tag="stats")
                        for s in range(2):
                            nc.vector.bn_stats(
                                out=stats[:, s, :], in_=xs[:, s * 512 : (s + 1) * 512])
                        mv = wk.tile([128, 2], F32, tag="mv")
                        nc.vector.bn_aggr(out=mv, in_=stats)
                        rstd = wk.tile([128, 1], F32, tag="rstd", bufs=2)
                        nc.scalar.activation(
                            out=rstd, in_=mv[:, 1:2], func=AF.Sqrt,
                            bias=eps_t, scale=1.0)
                        nc.vector.reciprocal(out=rstd, in_=rstd)
                        xn = wk.tile([128, Fd], F32, tag="xn", bufs=2)
                        rstd_b = rstd.to_broadcast([128, Fd])
                        nc.vector.scalar_tensor_tensor(
                            out=xn, in0=xs, scalar=mv[:, 0:1], in1=rstd_b,
                            op0=OP.subtract, op1=OP.mult)
                        if use_gb:
                            nc.vector.scalar_tensor_tensor(
                                out=xn, in0=xn, scalar=1.0, in1=gam_b,
                                op0=OP.mult, op1=OP.mult)
                            nc.vector.tensor_add(xn, xn, bet_b)
                        with tc.tile_pool(name=f"ps_tr{b}", bufs=4,
                                          space="PSUM") as ptr:
                            for s in range(2):
                                nc.scalar.activation(
                                    out=xa[:, b, s * 512 : (s + 1) * 512],
                                    in_=xn[:, s * 512 : (s + 1) * 512],
                                    func=AF.Tanh)
                                for f in range(4 * s, 4 * s + 4):
                                    pt = ptr.tile([128, 128], BF16, tag="tr")
                                    nc.tensor.transpose(
                                        pt, xa[:, b, f * 128 : (f + 1) * 128],
                                        ident)
                                    nc.scalar.activation(
                                        out=xT[:, f, b * 128 : (b + 1) * 128],
                                        in_=pt, func=AF.Copy)

            # ============ lstm-persistent fp8 weights ============
            with tc.tile_pool(name="lstmw", bufs=1) as lw:
                whh8_sb = lw.tile([128, KH, 4 * H], E4)   # 32KB/part
                whh8_r = whh8_d.ap().rearrange("(ko p) n -> p ko n", p=128)
                whh_r = whh_d.ap().rearrange("(ko p) n -> p ko n", p=128)

                # ===== Phase 4: pre^T = W_ih^T x^T + bsum =====
                with (
                    tc.tile_pool(name="wihstream", bufs=2) as ws2,
                    tc.tile_pool(name="ps_pre", bufs=2, space="PSUM") as ppr,
                ):
                    m_order = (list(range(0, 8)) + list(range(16, 32))
                               + list(range(8, 16)))
                    for idx, m in enumerate(m_order):
                        wm = ws2.tile([128, KH, 128], BF16, tag="wih", bufs=4)
                        nc.sync.dma_start(
                            wm, wih_d.ap()[m].rearrange("p (k j) -> p k j", j=128))
                        if idx % 4 == 0:
                            k8 = idx // 4
                            nc.sync.dma_start(whh_sb[:, k8, :], whh_r[:, k8, :])
                            if k8 == 0:
                                nc.sync.dma_start(
                                    w1_sb,
                                    w1_d.ap().rearrange("(ko p) n -> p ko n", p=128))
                                nc.sync.dma_start(
                                    w2_sb,
                                    w2_d.ap().rearrange("(ko p) n -> p ko n", p=128))
                        elif idx % 4 == 2:
                            k8 = idx // 4
                            nc.sync.dma_start(whh8_sb[:, k8, :], whh8_r[:, k8, :])
                        ps = ppr.tile([128, BS], F32, tag="pre")
                        for k in range(KH):
                            nc.tensor.matmul(
                                ps, wm[:, k, :], xT[:, k, :],
                                start=(k == 0), stop=(k == KH - 1))
                        nc.vector.tensor_scalar_add(
                            preT[:, m, :], ps, bsum_sb[:, m : m + 1])

                # ============ Phase 5: LSTM steps ============
                # Delta-fp8 recurrence with the MLP head DELAYED one step: the
                # PE order per step is [gate matmuls(t)][head matmuls(t-1)], so
                # the head fills the PE while step t's elementwise tail
                # (gates accum -> acts -> cell -> d8) completes. Cell state and
                # all tail intermediates are bf16 for 2x DVE throughput.
                with (
                    tc.tile_pool(name="acts", bufs=2) as ac,
                    tc.tile_pool(name="ps_g", bufs=2, space="PSUM") as psg,
                    tc.tile_pool(name="ps_m", bufs=1, space="PSUM") as psm,
                    tc.tile_pool(name="ps_w2", bufs=1, space="PSUM") as psw,
                ):
                    # group layout: gate q (i,f,g,o), H-half hh (0,1)
                    # m-tiles of group (q, hh): m = 8q + 4*hh + [0..4)
                    def gate_group_matmuls_bf16(q, hh, h_cur):
                        P = psg.tile([128, 4, BS], F32, tag="g")
                        for mi in range(4):
                            m = 8 * q + 4 * hh + mi
                            for kk in range(KH):
                                k = (kk + m) % KH
                                nc.tensor.matmul(
                                    P[:, mi, :],
                                    whh_sb[:, k, m * 128 : (m + 1) * 128],
                                    h_cur[:, k, :],
                                    start=(kk == 0), stop=(kk == KH - 1))
                        return P

                    def gate_group_matmuls_fp8(q, hh, d8):
                        P = psg.tile([128, 4, BS], F32, tag="g")
                        for mi in range(4):
                            m = 8 * q + 4 * hh + mi
                            for kp in range(KH // 2):
                                k = (kp + m) % (KH // 2)
                                nc.tensor.matmul(
                                    P[:, mi, :],
                                    whh8_sb[:, 2 * k : 2 * k + 2,
                                            m * 128 : (m + 1) * 128],
                                    d8[:, 2 * k : 2 * k + 2, :],
                                    start=(kp == 0), stop=(kp == KH // 2 - 1),
                                    perf_mode=DR)
                        return P

                    def half_cell(hh, si, sf, tg, so, first):
                        j0 = 4 * hh
                        csl = c_st[:, j0 : j0 + 4, :]
                        if first:
                            nc.vector.tensor_mul(csl, si, tg)
                        else:
                            t1 = ac.tile([128, 4, BS], BF16, tag="t1")
                            nc.vector.tensor_mul(t1, si, tg)
                            nc.vector.tensor_mul(csl, csl, sf)
                            nc.vector.tensor_add(csl, csl, t1)
                        tcn = ac.tile([128, 4, BS], BF16, tag="tc")
                        nc.scalar.activation(out=tcn, in_=csl, func=AF.Tanh)
                        nc.vector.tensor_mul(h_new[:, j0 : j0 + 4, :], so, tcn)

                    def half_d8(hh, t_next, h_cur, d8):
                        j0 = 4 * hh
                        dtmp = ac.tile([128, 4, BS], BF16, tag="dt")
                        nc.vector.tensor_sub(
                            dtmp, h_new[:, j0 : j0 + 4, :],
                            h_cur[:, j0 : j0 + 4, :])
                        nc.scalar.activation(
                            out=d8[:, j0 : j0 + 4, :], in_=dtmp, func=AF.Copy,
                            scale=SCALE[t_next])

                    def mlp_head(t, h_src):
                        Pm = psm.tile([128, H2 // 128, BS], F32, tag="m1")
                        for m in range(H2 // 128):
                            for k in range(KH):
                                nc.tensor.matmul(
                                    Pm[:, m, :],
                                    w1_sb[:, k, m * 128 : (m + 1) * 128],
                                    h_src[:, k, :],
                                    start=(k == 0), stop=(k == KH - 1))
                        for m in range(H2 // 128):
                            nc.scalar.activation(
                                out=relu1T[:, m, :], in_=Pm[:, m, :],
                                func=AF.Relu, bias=b1_sb[:, m : m + 1], scale=1.0)
                        ps2 = psw.tile([A, BS], F32, tag="w2")
                        for k2 in range(H2 // 128):
                            nc.tensor.matmul(
                                ps2, w2_sb[:, k2, :], relu1T[:, k2, :],
                                start=(k2 == 0), stop=(k2 == H2 // 128 - 1))
                        stage = ac.tile([A, BS], F32, tag="mu")
                        nc.scalar.activation(
                            out=stage, in_=ps2, func=AF.Tanh, bias=b2_t,
                            scale=1.0)
                        nc.sync.dma_start(mu_d.ap()[t], stage)

                    # ---- step 0: h0 = c0 = 0 -> gates = pre ----
                    h_new = hT[0]
                    for hh in range(2):
                        j0 = 4 * hh
                        si = ac.tile([128, 4, BS], BF16, tag="a0")
                        tg = ac.tile([128, 4, BS], BF16, tag="a2")
                        so = ac.tile([128, 4, BS], BF16, tag="a3")
                        nc.scalar.activation(
                            out=si, in_=preT[:, j0 : j0 + 4, :], func=AF.Sigmoid)
                        nc.scalar.activation(
                            out=tg, in_=preT[:, 16 + j0 : 16 + j0 + 4, :],
                            func=AF.Tanh)
                        nc.scalar.activation(
                            out=so, in_=preT[:, 24 + j0 : 24 + j0 + 4, :],
                            func=AF.Sigmoid)
                        half_cell(hh, si, None, tg, so, first=True)
                    d8_cur = None

                    # ---- steps 1..15 (head for t-1 emitted inside step t) ----
                    for t in range(1, T):
                        h_cur = hT[(t + 1) % 2]
                        h_new = hT[t % 2]
                        is_refresh = t in REFRESH
                        if not is_refresh:
                            dsc = 1.0 / (SW * SCALE[t])
                        need_d8 = (t + 1 < T) and ((t + 1) not in REFRESH)
                        d8_next = None
                        if need_d8:
                            d8_next = lw.tile([128, KH, BS], E4, tag="d8", bufs=2)
                        for hh in range(2):
                            j0 = 4 * hh
                            acts = {}
                            for q in range(4):
                                g0 = 8 * q + j0
                                gsl = gates[:, g0 : g0 + 4, :]
                                if is_refresh:
                                    P = gate_group_matmuls_bf16(q, hh, h_cur)
                                    nc.vector.tensor_add(
                                        gsl, P, preT[:, g0 : g0 + 4, :])
                                else:
                                    P = gate_group_matmuls_fp8(q, hh, d8_cur)
                                    nc.vector.scalar_tensor_tensor(
                                        out=gsl, in0=P, scalar=dsc, in1=gsl,
                                        op0=OP.mult, op1=OP.add)
                                out_a = ac.tile([128, 4, BS], BF16, tag=f"a{q}")
                                nc.scalar.activation(
                                    out=out_a, in_=gsl,
                                    func=AF.Tanh if q == 2 else AF.Sigmoid)
                                acts[q] = out_a
                            half_cell(hh, acts[0], acts[1], acts[2], acts[3],
                                      first=False)
                            if need_d8:
                                half_d8(hh, t + 1, h_cur, d8_next)
                        mlp_head(t - 1, h_cur)
                        d8_cur = d8_next
                    mlp_head(T - 1, hT[(T - 1) % 2])

    nc.compile()
    return nc


def kernel(**inputs):
    obs = np.asarray(inputs["obs"], np.float32)
    W_trunk = np.asarray(inputs["W_trunk"], np.float32)
    b_trunk = np.asarray(inputs["b_trunk"], np.float32)
    gamma = np.asarray(inputs["gamma"], np.float32)
    beta = np.asarray(inputs["beta"], np.float32)
    W_ih = np.asarray(inputs["W_ih"], np.float32)
    b_ih = np.asarray(inputs["b_ih"], np.float32)
    W_hh = np.asarray(inputs["W_hh"], np.float32)
    b_hh = np.asarray(inputs["b_hh"], np.float32)
    W1 = np.asarray(inputs["W1"], np.float32)
    b1 = np.asarray(inputs["b1"], np.float32)
    W2 = np.asarray(inputs["W2"], np.float32)
    b2 = np.asarray(inputs["b2"], np.float32)
    num_actions = int(np.asarray(inputs["num_actions"]))
    assert num_actions == T, f"kernel hardcodes T={T}, got {num_actions}"
    assert obs.shape == (B, R)

    use_btr = bool(b_trunk.any())
    use_gb = bool((gamma != 1).any() or beta.any())
    key = (use_btr, use_gb)
    if key not in _CACHE:
        _CACHE[key] = _build(use_btr, use_gb)
    nc = _CACHE[key]

    wtr = np.zeros((RP, Fd), BF)
    wtr[:R] = W_trunk.astype(BF)
    wih = np.ascontiguousarray(
        W_ih.astype(BF).reshape(KH, 128, M4, 128).transpose(2, 1, 0, 3)
    ).reshape(M4, 128, KH * 128)
    whh = W_hh.astype(BF)
    whh8 = (W_hh * SW).astype(E4NP)
    w1 = W1.astype(BF)
    w2 = W2.astype(BF)
    bsum = (b_ih + b_hh).astype(np.float32)

    in_maps = []
    for i in range(NC_):
        sh = obs[i * BS : (i + 1) * BS]           # [256, R]
        obsT = np.zeros((RP, BS), BF)
        obsT[:R] = np.ascontiguousarray(sh.T).astype(BF)
        m = {
            "obsT": obsT, "wtr": wtr, "wih": wih, "whh": whh, "whh8": whh8,
            "w1": w1, "w2": w2, "bsum": bsum, "b1": b1, "b2": b2,
        }
        if use_btr:
            m["btr"] = b_trunk
        if use_gb:
            m["gam"] = gamma
            m["bet"] = beta
        in_maps.append(m)

    res = bass_utils.run_bass_kernel_spmd(
        nc, in_maps, core_ids=list(range(NC_)),
        trace=bool(int(__import__("os").environ.get("KTRACE", "0"))),
    )
    _CACHE["last_result"] = res
    # per-core mu is [T, A, BS]; assemble to [B, T, A]
    full = np.concatenate([res.results[i]["mu"] for i in range(NC_)], axis=2)
    out = np.ascontiguousarray(np.transpose(full, (2, 0, 1)))
    return out


# revision 16
# speedup vs baseline: 1.1552x; 1.0549x over previous
"""Trainium2 Bass kernel for nn_LSTMActor: trunk GEMM -> LayerNorm -> Tanh ->
LSTM (16 steps, constant input) -> MLP head -> tanh.

Sharding: data-parallel over batch B=2048 across 8 cores (256 rows each);
weights replicated. Transposed layout throughout (feature dim on partitions).

Perf scheme vs plain bf16:
- LSTM recurrent matmul uses delta-sigma fp8: on "delta" steps the gates are
  updated incrementally, gates += W8^T @ fp8(h_t - h_{t-1}), with the fp8
  DoubleRow perf mode (2 k-subtiles per instruction = 2x matmul throughput).
  Quantization error is relative to |dh| (small and shrinking), and the fixed
  W8 quantization error telescopes across steps; full-bf16 "refresh" steps at
  t in {1,5} bound the residual W-side error window. Measured end-to-end
  error ~9.8e-3 vs the 2e-2 budget.
- The MLP head for step t-1 is emitted after step t's gate matmuls, filling
  the PE while the elementwise tail (gates accum -> acts -> cell -> d8) runs.
- Elementwise work runs in fused multi-tile ops (gate groups of 4 m-tiles,
  H-halves for the cell, bf16 intermediates for 2x DVE rate).
- W2 head runs transposed (stationary = W2, 6 output partitions) and mu is
  DMA'd out per step as [T, A, BS]; the host reassembles [B, T, A].
"""

import numpy as np
import ml_dtypes

import concourse.bass as bass
import concourse.tile as tile
from concourse import mybir, bacc
from concourse import bass_utils
from concourse.masks import make_identity

BF = ml_dtypes.bfloat16
E4NP = ml_dtypes.float8_e4m3
F32 = mybir.dt.float32
BF16 = mybir.dt.bfloat16
E4 = mybir.dt.float8e4
DR = mybir.MatmulPerfMode.DoubleRow
AluOp = None  # set in _build

B, R, Fd, H, A, T = 2048, 39200, 1024, 1024, 6, 16
NC_ = 8
BS = B // NC_          # 256 rows per core
NB = BS // 128         # 2 b-tiles per core
KT = 128
RP = ((R + KT - 1) // KT) * KT   # 39296
NK = RP // KT          # 307 K-tiles for trunk
KH = H // 128          # 8 K-tiles for H-dim GEMMs
M4 = 4 * H // 128      # 32 M-tiles of gates
H2 = H // 2            # 512
KG = 2                 # trunk K-tiles per DMA batch

REFRESH = (1, 5)
SW = 64.0              # W_hh fp8 scale
# measured max|h_t - h_{t-1}| of the bf16 trajectory, x1.25 safety
_DMAX = {2: 0.247, 3: 0.163, 4: 0.136, 5: 0.113, 6: 0.098, 7: 0.084,
         8: 0.072, 9: 0.057, 10: 0.049, 11: 0.0445, 12: 0.0435, 13: 0.041,
         14: 0.038, 15: 0.035}
SCALE = {t: float(2.0 ** np.floor(np.log2(48.0 / (d * 1.25))))
         for t, d in _DMAX.items()}

_CACHE = {}


def _build(use_btr, use_gb):
    nc = bacc.Bacc("TRN2", target_bir_lowering=False, debug=False)
    AF = mybir.ActivationFunctionType
    OP = mybir.AluOpType

    obsT_d = nc.dram_tensor("obsT", [RP, BS], BF16, kind="ExternalInput")
    wtr_d = nc.dram_tensor("wtr", [RP, Fd], BF16, kind="ExternalInput")
    wih_d = nc.dram_tensor("wih", [M4, 128, KH * 128], BF16, kind="ExternalInput")
    whh_d = nc.dram_tensor("whh", [H, 4 * H], BF16, kind="ExternalInput")
    whh8_d = nc.dram_tensor("whh8", [H, 4 * H], E4, kind="ExternalInput")
    w1_d = nc.dram_tensor("w1", [H, H2], BF16, kind="ExternalInput")
    w2_d = nc.dram_tensor("w2", [H2, A], BF16, kind="ExternalInput")
    bsum_d = nc.dram_tensor("bsum", [4 * H], F32, kind="ExternalInput")
    b1_d = nc.dram_tensor("b1", [H2], F32, kind="ExternalInput")
    b2_d = nc.dram_tensor("b2", [A], F32, kind="ExternalInput")
    if use_btr:
        btr_d = nc.dram_tensor("btr", [Fd], F32, kind="ExternalInput")
    if use_gb:
        gam_d = nc.dram_tensor("gam", [Fd], F32, kind="ExternalInput")
        bet_d = nc.dram_tensor("bet", [Fd], F32, kind="ExternalInput")
    mu_d = nc.dram_tensor("mu", [T, A, BS], F32, kind="ExternalOutput")

    def bc(ap1d, p=128):
        return bass.AP(tensor=ap1d.tensor, offset=ap1d.offset,
                       ap=[[0, p]] + [list(x) for x in ap1d.ap])

    with tile.TileContext(nc) as tc:
        with (
            tc.tile_pool(name="const", bufs=1) as cst,
            tc.tile_pool(name="state", bufs=1) as st,
        ):
            ident = cst.tile([128, 128], BF16)
            whh_sb = cst.tile([128, KH, 4 * H], BF16)     # 64KB/part
            w1_sb = cst.tile([128, KH, H2], BF16)         # 8KB/part
            w2_sb = cst.tile([128, H2 // 128, A], BF16)
            bsum_sb = cst.tile([128, M4], F32)
            b1_sb = cst.tile([128, H2 // 128], F32)
            b2_t = cst.tile([A, 1], F32)
            eps_t = cst.tile([128, 1], F32)

            xT = st.tile([128, KH, BS], BF16)
            preT = st.tile([128, M4, BS], BF16)           # 16KB/part
            gates = st.tile([128, M4, BS], BF16)          # 16KB/part
            c_st = st.tile([128, KH, BS], BF16)           # 4KB/part
            hT = [st.tile([128, KH, BS], BF16, name=f"hT{i}", tag=f"h{i}")
                  for i in range(2)]
            relu1T = st.tile([128, H2 // 128, BS], BF16)

            wtr_r = wtr_d.ap().rearrange("(ko p) n -> p ko n", p=128)
            obsT_r = obsT_d.ap().rearrange("(ko p) b -> p ko b", p=128)

            # ================= Phase 1: trunk GEMM =================
            with tc.tile_pool(name="ps_trunk", bufs=1, space="PSUM") as pst:
                psx = pst.tile([128, NB, Fd], F32)        # 8KB/part = 4 banks
                with tc.tile_pool(name="wstream", bufs=2) as ws:
                    for kg in range(0, NK, KG):
                        kn = min(KG, NK - kg)
                        wt = ws.tile([128, KG, Fd], BF16, tag="wtr", bufs=8)
                        ot = ws.tile([128, KG, BS], BF16, tag="obsT", bufs=6)
                        nc.sync.dma_start(wt[:, :kn, :], wtr_r[:, kg : kg + kn, :])
                        nc.sync.dma_start(ot[:, :kn, :], obsT_r[:, kg : kg + kn, :])
                        if kg == KG:
                            # small consts after the first trunk chunks queued
                            nc.sync.dma_start(
                                bsum_sb, bsum_d.ap().rearrange("(m p) -> p m", p=128))
                            nc.sync.dma_start(
                                b1_sb, b1_d.ap().rearrange("(m p) -> p m", p=128))
                            nc.sync.dma_start(
                                b2_t, b2_d.ap().rearrange("(a one) -> a one", one=1))
                            nc.vector.memset(eps_t, 1e-5)
                            make_identity(nc, ident)
                        for kk in range(kn):
                            k = kg + kk
                            for b in range(NB):
                                lhsT = ot[:, kk, b * 128 : (b + 1) * 128]
                                for n in range(2):
                                    nc.tensor.matmul(
                                        psx[:, b, n * 512 : (n + 1) * 512],
                                        lhsT,
                                        wt[:, kk, n * 512 : (n + 1) * 512],
                                        start=(k == 0),
                                        stop=(k == NK - 1),
                                    )

                # ============ Phase 2: LayerNorm + tanh ============
                with tc.tile_pool(name="lnwork", bufs=1) as wk:
                    if use_btr:
                        btr_b = wk.tile([128, Fd], F32, tag="btr")
                        nc.sync.dma_start(btr_b, bc(btr_d.ap()))
                    if use_gb:
                        gam_b = wk.tile([128, Fd], F32, tag="gam")
                        bet_b = wk.tile([128, Fd], F32, tag="bet")
                        nc.sync.dma_start(gam_b, bc(gam_d.ap()))
                        nc.sync.dma_start(bet_b, bc(bet_d.ap()))
                    xa = wk.tile([128, NB, Fd], BF16, tag="xa")
                    for b in range(NB):
                        if use_btr:
                            xs = wk.tile([128, Fd], F32, tag="xs", bufs=2)
                            nc.vector.tensor_add(xs, psx[:, b, :], btr_b)
                        else:
                            xs = psx[:, b, :]
                        stats = wk.tile([128, 2, 6], F32, tag="stats")
                        for s in range(2):
                            nc.vector.bn_stats(
                                out=stats[:, s, :], in_=xs[:, s * 512 : (s + 1) * 512])
                        mv = wk.tile([128, 2], F32, tag="mv")
                        nc.vector.bn_aggr(out=mv, in_=stats)
                        rstd = wk.tile([128, 1], F32, tag="rstd", bufs=2)
                        nc.scalar.activation(
                            out=rstd, in_=mv[:, 1:2], func=AF.Sqrt,
                            bias=eps_t, scale=1.0)
                        nc.vector.reciprocal(out=rstd, in_=rstd)
                        xn = wk.tile([128, Fd], F32, tag="xn", bufs=2)
                        rstd_b = rstd.to_broadcast([128, Fd])
                        nc.vector.scalar_tensor_tensor(
                            out=xn, in0=xs, scalar=mv[:, 0:1], in1=rstd_b,
                            op0=OP.subtract, op1=OP.mult)
                        if use_gb:
                            nc.vector.scalar_tensor_tensor(
                                out=xn, in0=xn, scalar=1.0, in1=gam_b,
                                op0=OP.mult, op1=OP.mult)
                            nc.vector.tensor_add(xn, xn, bet_b)
                        with tc.tile_pool(name=f"ps_tr{b}", bufs=4,
                                          space="PSUM") as ptr:
                            for s in range(2):
                                nc.scalar.activation(
                                    out=xa[:, b, s * 512 : (s + 1) * 512],
                                    in_=xn[:, s * 512 : (s + 1) * 512],
                                    func=AF.Tanh)
                                for f in range(4 * s, 4 * s + 4):
                                    pt = ptr.tile([128, 128], BF16, tag="tr")
                                    nc.tensor.transpose(
                                        pt, xa[:, b, f * 128 : (f + 1) * 128],
                                        ident)
                                    nc.scalar.activation(
                                        out=xT[:, f, b * 128 : (b + 1) * 128],
                                        in_=pt, func=AF.Copy)

            # ============ lstm-persistent fp8 weights ============
            with tc.tile_pool(name="lstmw", bufs=1) as lw:
                whh8_sb = lw.tile([128, KH, 4 * H], E4)   # 32KB/part
                whh8_r = whh8_d.ap().rearrange("(ko p) n -> p ko n", p=128)
                whh_r = whh_d.ap().rearrange("(ko p) n -> p ko n", p=128)

                # ===== Phase 4: pre^T = W_ih^T x^T + bsum =====
                with (
                    tc.tile_pool(name="wihstream", bufs=2) as ws2,
                    tc.tile_pool(name="ps_pre", bufs=2, space="PSUM") as ppr,
                ):
                    m_order = (list(range(0, 8)) + list(range(16, 32))
                               + list(range(8, 16)))
                    for idx, m in enumerate(m_order):
                        wm = ws2.tile([128, KH, 128], BF16, tag="wih", bufs=4)
                        nc.sync.dma_start(
                            wm, wih_d.ap()[m].rearrange("p (k j) -> p k j", j=128))
                        if idx % 4 == 0:
                            k8 = idx // 4
                            nc.sync.dma_start(whh_sb[:, k8, :], whh_r[:, k8, :])
                            if k8 == 0:
                                nc.sync.dma_start(
                                    w1_sb,
                                    w1_d.ap().rearrange("(ko p) n -> p ko n", p=128))
                                nc.sync.dma_start(
                                    w2_sb,
                                    w2_d.ap().rearrange("(ko p) n -> p ko n", p=128))
                        elif idx % 4 == 2:
                            k8 = idx // 4
                            nc.sync.dma_start(whh8_sb[:, k8, :], whh8_r[:, k8, :])
                        ps = ppr.tile([128, BS], F32, tag="pre")
                        for k in range(KH):
                            nc.tensor.matmul(
                                ps, wm[:, k, :], xT[:, k, :],
                                start=(k == 0), stop=(k == KH - 1))
                        nc.vector.tensor_scalar_add(
                            preT[:, m, :], ps, bsum_sb[:, m : m + 1])

                # ============ Phase 5: LSTM steps ============
                # Delta-fp8 recurrence with the MLP head DELAYED one step: the
                # PE order per step is [gate matmuls(t)][head matmuls(t-1)], so
                # the head fills the PE while step t's elementwise tail
                # (gates accum -> acts -> cell -> d8) completes. Cell state and
                # all tail intermediates are bf16 for 2x DVE throughput.
                with (
                    tc.tile_pool(name="acts", bufs=2) as ac,
                    tc.tile_pool(name="ps_g", bufs=2, space="PSUM") as psg,
                    tc.tile_pool(name="ps_m", bufs=1, space="PSUM") as psm,
                    tc.tile_pool(name="ps_w2", bufs=1, space="PSUM") as psw,
                ):
                    # group layout: gate q (i,f,g,o), H-half hh (0,1)
                    # m-tiles of group (q, hh): m = 8q + 4*hh + [0..4)
                    def gate_group_matmuls_bf16(q, hh, h_cur):
                        P = psg.tile([128, 4, BS], F32, tag="g")
                        for mi in range(4):
                            m = 8 * q + 4 * hh + mi
                            for kk in range(KH):
                                k = (kk + m) % KH
                                nc.tensor.matmul(
                                    P[:, mi, :],
                                    whh_sb[:, k, m * 128 : (m + 1) * 128],
                                    h_cur[:, k, :],
                                    start=(kk == 0), stop=(kk == KH - 1))
                        return P

                    def gate_group_matmuls_fp8(q, hh, d8):
                        P = psg.tile([128, 4, BS], F32, tag="g")
                        for mi in range(4):
                            m = 8 * q + 4 * hh + mi
                            for kp in range(KH // 2):
                                k = (kp + m) % (KH // 2)
                                nc.tensor.matmul(
                                    P[:, mi, :],
                                    whh8_sb[:, 2 * k : 2 * k + 2,
                                            m * 128 : (m + 1) * 128],
                                    d8[:, 2 * k : 2 * k + 2, :],
                                    start=(kp == 0), stop=(kp == KH // 2 - 1),
                                    perf_mode=DR)
                        return P

                    def half_cell(hh, si, sf, tg, so, first):
                        j0 = 4 * hh
                        csl = c_st[:, j0 : j0 + 4, :]
                        if first:
                            nc.vector.tensor_mul(csl, si, tg)
                        else:
                            t1 = ac.tile([128, 4, BS], BF16, tag="t1")
                            nc.vector.tensor_mul(t1, si, tg)
                            nc.vector.tensor_mul(csl, csl, sf)
                            nc.vector.tensor_add(csl, csl, t1)
                        tcn = ac.tile([128, 4, BS], BF16, tag="tc")
                        nc.scalar.activation(out=tcn, in_=csl, func=AF.Tanh)
                        nc.vector.tensor_mul(h_new[:, j0 : j0 + 4, :], so, tcn)

                    def half_d8(hh, t_next, h_cur, d8):
                        j0 = 4 * hh
                        dtmp = ac.tile([128, 4, BS], BF16, tag="dt")
                        nc.vector.tensor_sub(
                            dtmp, h_new[:, j0 : j0 + 4, :],
                            h_cur[:, j0 : j0 + 4, :])
                        nc.scalar.activation(
                            out=d8[:, j0 : j0 + 4, :], in_=dtmp, func=AF.Copy,
                            scale=SCALE[t_next])

                    def mlp_head(t, h_src):
                        Pm = psm.tile([128, H2 // 128, BS], F32, tag="m1")
                        for m in range(H2 // 128):
                            for k in range(KH):
                                nc.tensor.matmul(
                                    Pm[:, m, :],
                                    w1_sb[:, k, m * 128 : (m + 1) * 128],
                                    h_src[:, k, :],
                                    start=(k == 0), stop=(k == KH - 1))
                        for m in range(H2 // 128):
                            nc.scalar.activation(
                                out=relu1T[:, m, :], in_=Pm[:, m, :],
                                func=AF.Relu, bias=b1_sb[:, m : m + 1], scale=1.0)
                        ps2 = psw.tile([A, BS], F32, tag="w2")
                        for k2 in range(H2 // 128):
                            nc.tensor.matmul(
                                ps2, w2_sb[:, k2, :], relu1T[:, k2, :],
                                start=(k2 == 0), stop=(k2 == H2 // 128 - 1))
                        stage = ac.tile([A, BS], F32, tag="mu")
                        nc.scalar.activation(
                            out=stage, in_=ps2, func=AF.Tanh, bias=b2_t,
                            scale=1.0)
                        nc.sync.dma_start(mu_d.ap()[t], stage)

                    # ---- step 0: h0 = c0 = 0 -> gates = pre ----
                    h_new = hT[0]
                    for hh in range(2):
                        j0 = 4 * hh
                        si = ac.tile([128, 4, BS], BF16, tag="a0")
                        tg = ac.tile([128, 4, BS], BF16, tag="a2")
                        so = ac.tile([128, 4, BS], BF16, tag="a3")
                        nc.scalar.activation(
                            out=si, in_=preT[:, j0 : j0 + 4, :], func=AF.Sigmoid)
                        nc.scalar.activation(
                            out=tg, in_=preT[:, 16 + j0 : 16 + j0 + 4, :],
                            func=AF.Tanh)
                        nc.scalar.activation(
                            out=so, in_=preT[:, 24 + j0 : 24 + j0 + 4, :],
                            func=AF.Sigmoid)
                        half_cell(hh, si, None, tg, so, first=True)
                    d8_cur = None

                    # ---- steps 1..15 (head for t-1 emitted inside step t) ----
                    for t in range(1, T):
                        h_cur = hT[(t + 1) % 2]
                        h_new = hT[t % 2]
                        is_refresh = t in REFRESH
                        if not is_refresh:
                            dsc = 1.0 / (SW * SCALE[t])
                        need_d8 = (t + 1 < T) and ((t + 1) not in REFRESH)
                        d8_next = None
                        if need_d8:
                            d8_next = lw.tile([128, KH, BS], E4, tag="d8", bufs=2)
                        for hh in range(2):
                            j0 = 4 * hh
                            acts = {}
                            for q in range(4):
                                g0 = 8 * q + j0
                                gsl = gates[:, g0 : g0 + 4, :]
                                if is_refresh:
                                    P = gate_group_matmuls_bf16(q, hh, h_cur)
                                    nc.vector.tensor_add(
                                        gsl, P, preT[:, g0 : g0 + 4, :])
                                else:
                                    P = gate_group_matmuls_fp8(q, hh, d8_cur)
                                    nc.vector.scalar_tensor_tensor(
                                        out=gsl, in0=P, scalar=dsc, in1=gsl,
                                        op0=OP.mult, op1=OP.add)
                                out_a = ac.tile([128, 4, BS], BF16, tag=f"a{q}")
                                nc.scalar.activation(
                                    out=out_a, in_=gsl,
                                    func=AF.Tanh if q == 2 else AF.Sigmoid)
                                acts[q] = out_a
                            half_cell(hh, acts[0], acts[1], acts[2], acts[3],
                                      first=False)
                            if need_d8:
                                half_d8(hh, t + 1, h_cur, d8_next)
                        mlp_head(t - 1, h_cur)
                        d8_cur = d8_next
                    mlp_head(T - 1, hT[(T - 1) % 2])

    nc.compile()
    return nc


def kernel(**inputs):
    obs = np.asarray(inputs["obs"], np.float32)
    W_trunk = np.asarray(inputs["W_trunk"], np.float32)
    b_trunk = np.asarray(inputs["b_trunk"], np.float32)
    gamma = np.asarray(inputs["gamma"], np.float32)
    beta = np.asarray(inputs["beta"], np.float32)
    W_ih = np.asarray(inputs["W_ih"], np.float32)
    b_ih = np.asarray(inputs["b_ih"], np.float32)
    W_hh = np.asarray(inputs["W_hh"], np.float32)
    b_hh = np.asarray(inputs["b_hh"], np.float32)
    W1 = np.asarray(inputs["W1"], np.float32)
    b1 = np.asarray(inputs["b1"], np.float32)
    W2 = np.asarray(inputs["W2"], np.float32)
    b2 = np.asarray(inputs["b2"], np.float32)
    num_actions = int(np.asarray(inputs["num_actions"]))
    assert num_actions == T, f"kernel hardcodes T={T}, got {num_actions}"
    assert obs.shape == (B, R)

    use_btr = bool(b_trunk.any())
    use_gb = bool((gamma != 1).any() or beta.any())
    key = (use_btr, use_gb)
    if key not in _CACHE:
        _CACHE[key] = _build(use_btr, use_gb)
    nc = _CACHE[key]

    wtr = np.zeros((RP, Fd), BF)
    wtr[:R] = W_trunk.astype(BF)
    wih = np.ascontiguousarray(
        W_ih.astype(BF).reshape(KH, 128, M4, 128).transpose(2, 1, 0, 3)
    ).reshape(M4, 128, KH * 128)
    whh = W_hh.astype(BF)
    whh8 = (W_hh * SW).astype(E4NP)
    w1 = W1.astype(BF)
    w2 = W2.astype(BF)
    bsum = (b_ih + b_hh).astype(np.float32)

    in_maps = []
    for i in range(NC_):
        sh = obs[i * BS : (i + 1) * BS]           # [256, R]
        obsT = np.zeros((RP, BS), BF)
        obsT[:R] = np.ascontiguousarray(sh.T).astype(BF)
        m = {
            "obsT": obsT, "wtr": wtr, "wih": wih, "whh": whh, "whh8": whh8,
            "w1": w1, "w2": w2, "bsum": bsum, "b1": b1, "b2": b2,
        }
        if use_btr:
            m["btr"] = b_trunk
        if use_gb:
            m["gam"] = gamma
            m["bet"] = beta
        in_maps.append(m)

    res = bass_utils.run_bass_kernel_spmd(
        nc, in_maps, core_ids=list(range(NC_)),
        trace=bool(int(__import__("os").environ.get("KTRACE", "0"))),
    )
    _CACHE["last_result"] = res
    # per-core mu is [T, A, BS]; assemble to [B, T, A]
    full = np.concatenate([res.results[i]["mu"] for i in range(NC_)], axis=2)
    out = np.ascontiguousarray(np.transpose(full, (2, 0, 1)))
    return out


# revision 18
# speedup vs baseline: 1.1697x; 1.0126x over previous
"""Trainium2 Bass kernel for nn_LSTMActor: trunk GEMM -> LayerNorm -> Tanh ->
LSTM (16 steps, constant input) -> MLP head -> tanh.

Sharding: data-parallel over batch B=2048 across 8 cores (256 rows each);
weights replicated. Transposed layout throughout (feature dim on partitions).

Perf scheme vs plain bf16:
- LSTM recurrent matmul uses delta-sigma fp8: on "delta" steps the gates are
  updated incrementally, gates += W8^T @ fp8(h_t - h_{t-1}), with the fp8
  DoubleRow perf mode (2 k-subtiles per instruction = 2x matmul throughput).
  Quantization error is relative to |dh| (small and shrinking), and the fixed
  W8 quantization error telescopes across steps; full-bf16 "refresh" steps at
  t in {1,5} bound the residual W-side error window. Measured end-to-end
  error ~9.8e-3 vs the 2e-2 budget.
- The MLP head for step t-1 is emitted after step t's gate matmuls, filling
  the PE while the elementwise tail (gates accum -> acts -> cell -> d8) runs.
- Elementwise work runs in fused multi-tile ops (gate groups of 4 m-tiles,
  H-halves for the cell, bf16 intermediates for 2x DVE rate).
- W2 head runs transposed (stationary = W2, 6 output partitions) and mu is
  DMA'd out per step as [T, A, BS]; the host reassembles [B, T, A].
"""

import numpy as np
import ml_dtypes

import concourse.bass as bass
import concourse.tile as tile
from concourse import mybir, bacc
from concourse import bass_utils
from concourse.masks import make_identity

BF = ml_dtypes.bfloat16
E4NP = ml_dtypes.float8_e4m3
F32 = mybir.dt.float32
BF16 = mybir.dt.bfloat16
E4 = mybir.dt.float8e4
DR = mybir.MatmulPerfMode.DoubleRow
AluOp = None  # set in _build

B, R, Fd, H, A, T = 2048, 39200, 1024, 1024, 6, 16
NC_ = 8
BS = B // NC_          # 256 rows per core
NB = BS // 128         # 2 b-tiles per core
KT = 128
RP = ((R + KT - 1) // KT) * KT   # 39296
NK = RP // KT          # 307 K-tiles for trunk
KH = H // 128          # 8 K-tiles for H-dim GEMMs
M4 = 4 * H // 128      # 32 M-tiles of gates
H2 = H // 2            # 512
KG = 2                 # trunk K-tiles per DMA batch

REFRESH = (1, 5)
SW = 64.0              # W_hh fp8 scale
# measured max|h_t - h_{t-1}| of the bf16 trajectory, x1.25 safety
_DMAX = {2: 0.247, 3: 0.163, 4: 0.136, 5: 0.113, 6: 0.098, 7: 0.084,
         8: 0.072, 9: 0.057, 10: 0.049, 11: 0.0445, 12: 0.0435, 13: 0.041,
         14: 0.038, 15: 0.035}
SCALE = {t: float(2.0 ** np.floor(np.log2(48.0 / (d * 1.25))))
         for t, d in _DMAX.items()}

_CACHE = {}


def _build(use_btr, use_gb):
    nc = bacc.Bacc("TRN2", target_bir_lowering=False, debug=False)
    AF = mybir.ActivationFunctionType
    OP = mybir.AluOpType

    obsT_d = nc.dram_tensor("obsT", [RP, BS], BF16, kind="ExternalInput")
    wtr_d = nc.dram_tensor("wtr", [RP, Fd], BF16, kind="ExternalInput")
    wih_d = nc.dram_tensor("wih", [M4, 128, KH * 128], BF16, kind="ExternalInput")
    whh_d = nc.dram_tensor("whh", [H, 4 * H], BF16, kind="ExternalInput")
    whh8_d = nc.dram_tensor("whh8", [H, 4 * H], E4, kind="ExternalInput")
    w1_d = nc.dram_tensor("w1", [H, H2], BF16, kind="ExternalInput")
    w2_d = nc.dram_tensor("w2", [H2, A], BF16, kind="ExternalInput")
    bsum_d = nc.dram_tensor("bsum", [4 * H], F32, kind="ExternalInput")
    b1_d = nc.dram_tensor("b1", [H2], F32, kind="ExternalInput")
    b2_d = nc.dram_tensor("b2", [A], F32, kind="ExternalInput")
    if use_btr:
        btr_d = nc.dram_tensor("btr", [Fd], F32, kind="ExternalInput")
    if use_gb:
        gam_d = nc.dram_tensor("gam", [Fd], F32, kind="ExternalInput")
        bet_d = nc.dram_tensor("bet", [Fd], F32, kind="ExternalInput")
    mu_d = nc.dram_tensor("mu", [T, A, BS], F32, kind="ExternalOutput")

    def bc(ap1d, p=128):
        return bass.AP(tensor=ap1d.tensor, offset=ap1d.offset,
                       ap=[[0, p]] + [list(x) for x in ap1d.ap])

    with tile.TileContext(nc) as tc:
        with (
            tc.tile_pool(name="const", bufs=1) as cst,
            tc.tile_pool(name="state", bufs=1) as st,
        ):
            ident = cst.tile([128, 128], BF16)
            whh_sb = cst.tile([128, KH, 4 * H], BF16)     # 64KB/part
            w1_sb = cst.tile([128, KH, H2], BF16)         # 8KB/part
            w2_sb = cst.tile([128, H2 // 128, A], BF16)
            bsum_sb = cst.tile([128, M4], F32)
            b1_sb = cst.tile([128, H2 // 128], F32)
            b2_t = cst.tile([A, 1], F32)
            eps_t = cst.tile([128, 1], F32)

            xT = st.tile([128, KH, BS], BF16)
            preT = st.tile([128, M4, BS], BF16)           # 16KB/part
            gates = st.tile([128, M4, BS], BF16)          # 16KB/part
            c_st = st.tile([128, KH, BS], BF16)           # 4KB/part
            hT = [st.tile([128, KH, BS], BF16, name=f"hT{i}", tag=f"h{i}")
                  for i in range(2)]
            relu1T = st.tile([128, H2 // 128, BS], BF16)

            wtr_r = wtr_d.ap().rearrange("(ko p) n -> p ko n", p=128)
            obsT_r = obsT_d.ap().rearrange("(ko p) b -> p ko b", p=128)

            # ================= Phase 1: trunk GEMM =================
            with tc.tile_pool(name="ps_trunk", bufs=1, space="PSUM") as pst:
                psx = pst.tile([128, NB, Fd], F32)        # 8KB/part = 4 banks
                with tc.tile_pool(name="wstream", bufs=2) as ws:
                    for kg in range(0, NK, KG):
                        kn = min(KG, NK - kg)
                        wt = ws.tile([128, KG, Fd], BF16, tag="wtr", bufs=8)
                        ot = ws.tile([128, KG, BS], BF16, tag="obsT", bufs=6)
                        nc.sync.dma_start(wt[:, :kn, :], wtr_r[:, kg : kg + kn, :])
                        nc.sync.dma_start(ot[:, :kn, :], obsT_r[:, kg : kg + kn, :])
                        if kg == KG:
                            # small consts after the first trunk chunks queued
                            nc.sync.dma_start(
                                bsum_sb, bsum_d.ap().rearrange("(m p) -> p m", p=128))
                            nc.sync.dma_start(
                                b1_sb, b1_d.ap().rearrange("(m p) -> p m", p=128))
                            nc.sync.dma_start(
                                b2_t, b2_d.ap().rearrange("(a one) -> a one", one=1))
                            nc.vector.memset(eps_t, 1e-5)
                            make_identity(nc, ident)
                        for kk in range(kn):
                            k = kg + kk
                            for b in range(NB):
                                lhsT = ot[:, kk, b * 128 : (b + 1) * 128]
                                for n in range(2):
                                    nc.tensor.matmul(
                                        psx[:, b, n * 512 : (n + 1) * 512],
                                        lhsT,
                                        wt[:, kk, n * 512 : (n + 1) * 512],
                                        start=(k == 0),
                                        stop=(k == NK - 1),
                                    )

                # ============ Phase 2: LayerNorm + tanh ============
                with tc.tile_pool(name="lnwork", bufs=1) as wk:
                    if use_btr:
                        btr_b = wk.tile([128, Fd], F32, tag="btr")
                        nc.sync.dma_start(btr_b, bc(btr_d.ap()))
                    if use_gb:
                        gam_b = wk.tile([128, Fd], F32, tag="gam")
                        bet_b = wk.tile([128, Fd], F32, tag="bet")
                        nc.sync.dma_start(gam_b, bc(gam_d.ap()))
                        nc.sync.dma_start(bet_b, bc(bet_d.ap()))
                    xa = wk.tile([128, NB, Fd], BF16, tag="xa")
                    for b in range(NB):
                        if use_btr:
                            xs = wk.tile([128, Fd], F32, tag="xs", bufs=2)
                            nc.vector.tensor_add(xs, psx[:, b, :], btr_b)
                        else:
                            xs = psx[:, b, :]
                        stats = wk.tile([128, 2, 6], F32, tag="stats")
                        for s in range(2):
                            nc.vector.bn_stats(
                                out=stats[:, s, :], in_=xs[:, s * 512 : (s + 1) * 512])
                        mv = wk.tile([128, 2], F32, tag="mv")
                        nc.vector.bn_aggr(out=mv, in_=stats)
                        rstd = wk.tile([128, 1], F32, tag="rstd", bufs=2)
                        nc.scalar.activation(
                            out=rstd, in_=mv[:, 1:2], func=AF.Sqrt,
                            bias=eps_t, scale=1.0)
                        nc.vector.reciprocal(out=rstd, in_=rstd)
                        xn = wk.tile([128, Fd], F32, tag="xn", bufs=2)
                        rstd_b = rstd.to_broadcast([128, Fd])
                        nc.vector.scalar_tensor_tensor(
                            out=xn, in0=xs, scalar=mv[:, 0:1], in1=rstd_b,
                            op0=OP.subtract, op1=OP.mult)
                        if use_gb:
                            nc.vector.scalar_tensor_tensor(
                                out=xn, in0=xn, scalar=1.0, in1=gam_b,
                                op0=OP.mult, op1=OP.mult)
                            nc.vector.tensor_add(xn, xn, bet_b)
                        with tc.tile_pool(name=f"ps_tr{b}", bufs=4,
                                          space="PSUM") as ptr:
                            for s in range(2):
                                nc.scalar.activation(
                                    out=xa[:, b, s * 512 : (s + 1) * 512],
                                    in_=xn[:, s * 512 : (s + 1) * 512],
                                    func=AF.Tanh)
                                for f in range(4 * s, 4 * s + 4):
                                    pt = ptr.tile([128, 128], BF16, tag="tr")
                                    nc.tensor.transpose(
                                        pt, xa[:, b, f * 128 : (f + 1) * 128],
                                        ident)
                                    nc.scalar.activation(
                                        out=xT[:, f, b * 128 : (b + 1) * 128],
                                        in_=pt, func=AF.Copy)

            # ============ lstm-persistent fp8 weights ============
            with tc.tile_pool(name="lstmw", bufs=1) as lw:
                whh8_sb = lw.tile([128, KH, 4 * H], E4)   # 32KB/part
                whh8_r = whh8_d.ap().rearrange("(ko p) n -> p ko n", p=128)
                whh_r = whh_d.ap().rearrange("(ko p) n -> p ko n", p=128)

                # ===== Phase 4: pre^T = W_ih^T x^T + bsum =====
                with (
                    tc.tile_pool(name="wihstream", bufs=2) as ws2,
                    tc.tile_pool(name="ps_pre", bufs=2, space="PSUM") as ppr,
                ):
                    m_order = (list(range(0, 8)) + list(range(16, 32))
                               + list(range(8, 16)))
                    for idx, m in enumerate(m_order):
                        wm = ws2.tile([128, KH, 128], BF16, tag="wih", bufs=8)
                        nc.sync.dma_start(
                            wm, wih_d.ap()[m].rearrange("p (k j) -> p k j", j=128))
                        if idx % 4 == 0:
                            k8 = idx // 4
                            nc.sync.dma_start(whh_sb[:, k8, :], whh_r[:, k8, :])
                            if k8 == 0:
                                nc.sync.dma_start(
                                    w1_sb,
                                    w1_d.ap().rearrange("(ko p) n -> p ko n", p=128))
                                nc.sync.dma_start(
                                    w2_sb,
                                    w2_d.ap().rearrange("(ko p) n -> p ko n", p=128))
                        elif idx % 4 == 2:
                            k8 = idx // 4
                            nc.sync.dma_start(whh8_sb[:, k8, :], whh8_r[:, k8, :])
                        ps = ppr.tile([128, BS], F32, tag="pre")
                        for k in range(KH):
                            nc.tensor.matmul(
                                ps, wm[:, k, :], xT[:, k, :],
                                start=(k == 0), stop=(k == KH - 1))
                        nc.vector.tensor_scalar_add(
                            preT[:, m, :], ps, bsum_sb[:, m : m + 1])

                # ============ Phase 5: LSTM steps ============
                # Delta-fp8 recurrence with the MLP head DELAYED one step: the
                # PE order per step is [gate matmuls(t)][head matmuls(t-1)], so
                # the head fills the PE while step t's elementwise tail
                # (gates accum -> acts -> cell -> d8) completes. Cell state and
                # all tail intermediates are bf16 for 2x DVE throughput.
                with (
                    tc.tile_pool(name="acts", bufs=2) as ac,
                    tc.tile_pool(name="ps_g", bufs=2, space="PSUM") as psg,
                    tc.tile_pool(name="ps_m", bufs=1, space="PSUM") as psm,
                    tc.tile_pool(name="ps_w2", bufs=2, space="PSUM") as psw,
                ):
                    # group layout: gate q (i,f,g,o), H-half hh (0,1)
                    # m-tiles of group (q, hh): m = 8q + 4*hh + [0..4)
                    def gate_group_matmuls_bf16(q, hh, h_cur):
                        P = psg.tile([128, 4, BS], F32, tag="g")
                        for mi in range(4):
                            m = 8 * q + 4 * hh + mi
                            for k in range(KH):
                                kk = k
                                nc.tensor.matmul(
                                    P[:, mi, :],
                                    whh_sb[:, k, m * 128 : (m + 1) * 128],
                                    h_cur[:, k, :],
                                    start=(kk == 0), stop=(kk == KH - 1))
                        return P

                    def gate_group_matmuls_fp8(q, hh, d8):
                        P = psg.tile([128, 4, BS], F32, tag="g")
                        for mi in range(4):
                            m = 8 * q + 4 * hh + mi
                            for kp in range(KH // 2):
                                k = kp
                                nc.tensor.matmul(
                                    P[:, mi, :],
                                    whh8_sb[:, 2 * k : 2 * k + 2,
                                            m * 128 : (m + 1) * 128],
                                    d8[:, 2 * k : 2 * k + 2, :],
                                    start=(kp == 0), stop=(kp == KH // 2 - 1),
                                    perf_mode=DR)
                        return P

                    def half_cell(hh, si, sf, tg, so, first):
                        j0 = 4 * hh
                        csl = c_st[:, j0 : j0 + 4, :]
                        if first:
                            nc.vector.tensor_mul(csl, si, tg)
                        else:
                            t1 = ac.tile([128, 4, BS], BF16, tag="t1")
                            nc.vector.tensor_mul(t1, si, tg)
                            nc.vector.tensor_mul(csl, csl, sf)
                            nc.vector.tensor_add(csl, csl, t1)
                        tcn = ac.tile([128, 4, BS], BF16, tag="tc")
                        nc.scalar.activation(out=tcn, in_=csl, func=AF.Tanh)
                        nc.vector.tensor_mul(h_new[:, j0 : j0 + 4, :], so, tcn)

                    def half_d8(hh, t_next, h_cur, d8):
                        j0 = 4 * hh
                        dtmp = ac.tile([128, 4, BS], BF16, tag="dt")
                        nc.vector.tensor_sub(
                            dtmp, h_new[:, j0 : j0 + 4, :],
                            h_cur[:, j0 : j0 + 4, :])
                        nc.scalar.activation(
                            out=d8[:, j0 : j0 + 4, :], in_=dtmp, func=AF.Copy,
                            scale=SCALE[t_next])

                    def mlp_head(t, h_src):
                        Pm = psm.tile([128, H2 // 128, BS], F32, tag="m1")
                        for m in range(H2 // 128):
                            for k in range(KH):
                                nc.tensor.matmul(
                                    Pm[:, m, :],
                                    w1_sb[:, k, m * 128 : (m + 1) * 128],
                                    h_src[:, k, :],
                                    start=(k == 0), stop=(k == KH - 1))
                        for m in range(H2 // 128):
                            nc.scalar.activation(
                                out=relu1T[:, m, :], in_=Pm[:, m, :],
                                func=AF.Relu, bias=b1_sb[:, m : m + 1], scale=1.0)
                        ps2 = psw.tile([A, BS], F32, tag="w2")
                        for k2 in range(H2 // 128):
                            nc.tensor.matmul(
                                ps2, w2_sb[:, k2, :], relu1T[:, k2, :],
                                start=(k2 == 0), stop=(k2 == H2 // 128 - 1))
                        stage = ac.tile([A, BS], F32, tag="mu")
                        nc.scalar.activation(
                            out=stage, in_=ps2, func=AF.Tanh, bias=b2_t,
                            scale=1.0)
                        nc.sync.dma_start(mu_d.ap()[t], stage)

                    # ---- step 0: h0 = c0 = 0 -> gates = pre ----
                    h_new = hT[0]
                    for hh in range(2):
                        j0 = 4 * hh
                        si = ac.tile([128, 4, BS], BF16, tag="a0", bufs=3)
                        tg = ac.tile([128, 4, BS], BF16, tag="a2", bufs=3)
                        so = ac.tile([128, 4, BS], BF16, tag="a3", bufs=3)
                        nc.scalar.activation(
                            out=si, in_=preT[:, j0 : j0 + 4, :], func=AF.Sigmoid)
                        nc.scalar.activation(
                            out=tg, in_=preT[:, 16 + j0 : 16 + j0 + 4, :],
                            func=AF.Tanh)
                        nc.scalar.activation(
                            out=so, in_=preT[:, 24 + j0 : 24 + j0 + 4, :],
                            func=AF.Sigmoid)
                        half_cell(hh, si, None, tg, so, first=True)
                    d8_cur = None

                    # ---- steps 1..15 (head for t-1 emitted inside step t) ----
                    for t in range(1, T):
                        h_cur = hT[(t + 1) % 2]
                        h_new = hT[t % 2]
                        is_refresh = t in REFRESH
                        if not is_refresh:
                            dsc = 1.0 / (SW * SCALE[t])
                        need_d8 = (t + 1 < T) and ((t + 1) not in REFRESH)
                        d8_next = None
                        if need_d8:
                            d8_next = lw.tile([128, KH, BS], E4, tag="d8", bufs=2)
                        for hh in range(2):
                            j0 = 4 * hh
                            acts = {}
                            for q in range(4):
                                g0 = 8 * q + j0
                                gsl = gates[:, g0 : g0 + 4, :]
                                if is_refresh:
                                    P = gate_group_matmuls_bf16(q, hh, h_cur)
                                    nc.vector.tensor_add(
                                        gsl, P, preT[:, g0 : g0 + 4, :])
                                else:
                                    P = gate_group_matmuls_fp8(q, hh, d8_cur)
                                    nc.vector.scalar_tensor_tensor(
                                        out=gsl, in0=P, scalar=dsc, in1=gsl,
                                        op0=OP.mult, op1=OP.add)
                                out_a = ac.tile([128, 4, BS], BF16, tag=f"a{q}", bufs=3)
                                nc.scalar.activation(
                                    out=out_a, in_=gsl,
                                    func=AF.Tanh if q == 2 else AF.Sigmoid)
                                acts[q] = out_a
                            half_cell(hh, acts[0], acts[1], acts[2], acts[3],
                                      first=False)
                            if need_d8:
                                half_d8(hh, t + 1, h_cur, d8_next)
                        mlp_head(t - 1, h_cur)
                        d8_cur = d8_next
                    mlp_head(T - 1, hT[(T - 1) % 2])

    nc.compile()
    return nc


def kernel(**inputs):
    obs = np.asarray(inputs["obs"], np.float32)
    W_trunk = np.asarray(inputs["W_trunk"], np.float32)
    b_trunk = np.asarray(inputs["b_trunk"], np.float32)
    gamma = np.asarray(inputs["gamma"], np.float32)
    beta = np.asarray(inputs["beta"], np.float32)
    W_ih = np.asarray(inputs["W_ih"], np.float32)
    b_ih = np.asarray(inputs["b_ih"], np.float32)
    W_hh = np.asarray(inputs["W_hh"], np.float32)
    b_hh = np.asarray(inputs["b_hh"], np.float32)
    W1 = np.asarray(inputs["W1"], np.float32)
    b1 = np.asarray(inputs["b1"], np.float32)
    W2 = np.asarray(inputs["W2"], np.float32)
    b2 = np.asarray(inputs["b2"], np.float32)
    num_actions = int(np.asarray(inputs["num_actions"]))
    assert num_actions == T, f"kernel hardcodes T={T}, got {num_actions}"
    assert obs.shape == (B, R)

    use_btr = bool(b_trunk.any())
    use_gb = bool((gamma != 1).any() or beta.any())
    key = (use_btr, use_gb)
    if key not in _CACHE:
        _CACHE[key] = _build(use_btr, use_gb)
    nc = _CACHE[key]

    wtr = np.zeros((RP, Fd), BF)
    wtr[:R] = W_trunk.astype(BF)
    wih = np.ascontiguousarray(
        W_ih.astype(BF).reshape(KH, 128, M4, 128).transpose(2, 1, 0, 3)
    ).reshape(M4, 128, KH * 128)
    whh = W_hh.astype(BF)
    whh8 = (W_hh * SW).astype(E4NP)
    w1 = W1.astype(BF)
    w2 = W2.astype(BF)
    bsum = (b_ih + b_hh).astype(np.float32)

    in_maps = []
    for i in range(NC_):
        sh = obs[i * BS : (i + 1) * BS]           # [256, R]
        obsT = np.zeros((RP, BS), BF)
        obsT[:R] = np.ascontiguousarray(sh.T).astype(BF)
        m = {
            "obsT": obsT, "wtr": wtr, "wih": wih, "whh": whh, "whh8": whh8,
            "w1": w1, "w2": w2, "bsum": bsum, "b1": b1, "b2": b2,
        }
        if use_btr:
            m["btr"] = b_trunk
        if use_gb:
            m["gam"] = gamma
            m["bet"] = beta
        in_maps.append(m)

    res = bass_utils.run_bass_kernel_spmd(
        nc, in_maps, core_ids=list(range(NC_)),
        trace=bool(int(__import__("os").environ.get("KTRACE", "0"))),
    )
    _CACHE["last_result"] = res
    # per-core mu is [T, A, BS]; assemble to [B, T, A]
    full = np.concatenate([res.results[i]["mu"] for i in range(NC_)], axis=2)
    out = np.ascontiguousarray(np.transpose(full, (2, 0, 1)))
    return out
